# revision 20
# baseline (speedup 1.0000x reference)
"""Trainium2 Bass kernel for nn_Encoder (tri-modal Mamba encoder), v3.

kernel(**inputs) takes FULL unsharded numpy inputs and returns the FULL
output (B, W, 2N+E, D). Batch B=8 is sharded across 8 NeuronCores (pure
data parallel, no collectives); params are replicated.

v3 vs v2 (same math, new orchestration):
- Half-block (64-seq) processing units, software-pipelined end to end:
  the AddNorm+FFN+output phase (C) of each half-block is emitted as a
  persistent generator that drains into whatever later window has engine
  slack, so the serial phase-C tail is gone.
- LayerNorm stats finish per column-tile with rsqrt = exp(-0.5*ln(v+eps))
  so interleaved phases stay inside the natural_log_exp activation-table
  set (Silu windows are kept separate: A1 and mix).
- s / LN stats / LN scale factors live in SBUF; ym merged into s tiles.
- One shared 8-bank PSUM tag discipline (psA 2x[128,1024]f32,
  psY 2x[128,512], psC 2x[128,512]) across all phases.
- Output path: 4 batched f32 transposes per 512-col tile, one copy, one
  DMA (q t d scatter) straight from the LN-f apply.
"""

import ml_dtypes
import numpy as np
from contextlib import ExitStack

import concourse.bass as bass
import concourse.tile as tile
from concourse import bacc, mybir
from concourse.bass_utils import run_bass_kernel_spmd

D, DI, SS, KK, RR = 128, 256, 16, 4, 8
B, W, N, E = 8, 64, 128, 256
Q = 64                       # seqs per half-block
CBLK = Q * W                 # 4096 cols per half-block
CT = 512                     # column tile (8 seqs)
NT = CBLK // CT              # 8 tiles per half-block
NW = CBLK // 128             # 32 stat cols per partition
f32 = mybir.dt.float32
f32r = mybir.dt.float32r
bf16 = mybir.dt.bfloat16
AF = mybir.ActivationFunctionType
OP = mybir.AluOpType

# (name, modality, input key, q offset, output entity offset)
HBLOCKS = [("n0", 0, "x_n", 0, 0), ("n1", 0, "x_n", 64, 64),
           ("l0", 2, "x_l", 0, 384), ("l1", 2, "x_l", 64, 448),
           ("t0", 1, "x_t", 0, 128), ("t1", 1, "x_t", 64, 192),
           ("t2", 1, "x_t", 128, 256), ("t3", 1, "x_t", 192, 320)]
HB = {b[0]: b for b in HBLOCKS}
N_CORES = 8
LN_EPS = 1e-5


class Pack:
    def __init__(self):
        self.cols = []
        self.off = {}
        self.n = 0

    def add(self, name, arr, dtype=np.float32):
        arr = np.asarray(arr, dtype)
        assert arr.ndim == 2 and arr.shape[0] <= 128
        a = np.zeros((128, arr.shape[1]), dtype)
        a[: arr.shape[0]] = arr
        self.off[name] = (self.n, arr.shape[1])
        self.cols.append(a)
        self.n += arr.shape[1]

    def build(self):
        return np.concatenate(self.cols, axis=1)


def _host_pack(inp):
    """Returns (wp f32-staged-to-f32r, vp f32, bp bf16, flags)."""
    flags = {}
    dtb = np.asarray(inp["mp_dt_b"], np.float64)
    flags["dtb_const"] = float(dtb.flat[0]) if np.ptp(dtb) < 1e-12 else None
    flags["D_ones"] = bool(np.allclose(np.asarray(inp["mp_D"]), 1.0))
    flags["convb_zero"] = bool(np.all(np.asarray(inp["mp_conv_b"]) == 0.0))
    flags["f1b_zero"] = bool(np.all(np.asarray(inp["ff1_b"]) == 0.0))
    flags["f2b_zero"] = bool(np.all(np.asarray(inp["ff2_b"]) == 0.0))
    flags["an_id"] = bool(np.all(np.asarray(inp["an_g"]) == 1.0)
                          and np.all(np.asarray(inp["an_b"]) == 0.0))
    flags["fln_id"] = bool(np.all(np.asarray(inp["fln_g"]) == 1.0)
                           and np.all(np.asarray(inp["fln_b"]) == 0.0))
    flags["mixb_zero"] = bool(np.all(np.asarray(inp["mix_b"]) == 0.0))
    A = -np.exp(np.asarray(inp["mp_Alog"], np.float64))      # (3, DI, S)
    flags["A_shared"] = bool(
        np.ptp(A, axis=(0, 1)).max() < 1e-9 * np.abs(A).max())

    bp = Pack()   # bf16 weights
    for g in range(16):
        sm = np.zeros((128, 128), np.float32)
        for k in range(128):
            sm[k, g * 8 + k // 16] = 1.0
        bp.add(f"sum{g}", sm)

    def delta_A(Am, cc, g):
        dl = np.zeros((128, 128), np.float32)
        for j in range(128):
            dl[g * 8 + j // 16, j] = Am[cc * 128 + g * 8 + j // 16, j % 16]
        return dl

    if flags["A_shared"]:
        for g in range(16):
            bp.add(f"dA{g}", delta_A(A[0], 0, g))
    else:
        for m in range(3):
            for cc in range(2):
                for g in range(16):
                    bp.add(f"dA{m}{cc}{g}", delta_A(A[m], cc, g))

    wp = Pack()   # fp32 staged -> f32r on device
    vp = Pack()   # fp32 per-partition vectors
    for m in range(3):
        bp.add(f"win{m}", inp["mp_in"][m])                   # (D, 512)
        wxp = inp["mp_xproj"][m]                             # (DI, 40)
        for cc in range(2):
            bp.add(f"bc{m}{cc}", wxp[cc * 128:(cc + 1) * 128])
        dtw = inp["mp_dt_w"][m]                              # (R, DI)
        for cc in range(2):
            bp.add(f"dtw{m}{cc}", dtw[:, cc * 128:(cc + 1) * 128])
        wout = inp["mp_out"][m]                              # (DI, D)
        for cc in range(2):
            bp.add(f"wout{m}{cc}", wout[cc * 128:(cc + 1) * 128])
        bp.add(f"ff1{m}", inp["ff1_w"][m])                   # (D, 512)
        ff2 = inp["ff2_w"][m]                                # (4D, D)
        for c4 in range(4):
            bp.add(f"ff2{m}{c4}", ff2[c4 * 128:(c4 + 1) * 128])
    mixw = inp["mix_w"]
    for kc in range(2):
        for mc in range(2):
            bp.add(f"mix{kc}{mc}", mixw[kc * 128:(kc + 1) * 128,
                                        mc * 128:(mc + 1) * 128])
    wp.add("onesD", np.full((128, 1), 1.0 / D, np.float32))
    bp.add("onesDb", np.full((128, 1), 1.0 / D, np.float32))
    wp.add("ones1", np.ones((1, 128), np.float32))

    vp.add("eps", np.full((128, 1), LN_EPS, np.float32))
    if flags["dtb_const"] is not None:
        vp.add("dtbc", np.full((128, 1), flags["dtb_const"], np.float32))
    vp.add("I64", np.eye(64, dtype=np.float32))
    vp.add("I128", np.eye(128, dtype=np.float32))
    for m in range(3):
        cw = inp["mp_conv_w"][m]
        for cc in range(2):
            sl = slice(cc * 128, (cc + 1) * 128)
            vp.add(f"cw{m}{cc}", cw[sl])                     # 4 cols
            if not flags["convb_zero"]:
                vp.add(f"cb{m}{cc}", inp["mp_conv_b"][m][sl, None])
            if flags["dtb_const"] is None:
                vp.add(f"dtb{m}{cc}", inp["mp_dt_b"][m][sl, None])
            if not flags["D_ones"]:
                vp.add(f"Dp{m}{cc}", inp["mp_D"][m][sl, None])
        if not flags["f1b_zero"]:
            for c4 in range(4):
                vp.add(f"f1b{m}{c4}",
                       inp["ff1_b"][m][c4 * 128:(c4 + 1) * 128, None])
        if not flags["f2b_zero"]:
            vp.add(f"f2b{m}", inp["ff2_b"][m][:, None])
        if not flags["an_id"]:
            vp.add(f"ang{m}", inp["an_g"][m][:, None])
            vp.add(f"anb{m}", inp["an_b"][m][:, None])
        if not flags["fln_id"]:
            vp.add(f"flg{m}", inp["fln_g"][m][:, None])
            vp.add(f"flb{m}", inp["fln_b"][m][:, None])
    if not flags["mixb_zero"]:
        for mc in range(2):
            vp.add(f"mixb{mc}", inp["mix_b"][mc * 128:(mc + 1) * 128, None])
    return wp, vp, bp, flags


def _drain(g):
    for _ in g:
        pass


def _weave(streams):
    """streams: (gen, weight) pairs; round-robin to exhaustion."""
    live = [[iter(g), w] for g, w in streams]
    while live:
        for ent in list(live):
            g, w = ent
            for _ in range(w):
                try:
                    next(g)
                except StopIteration:
                    live.remove(ent)
                    break


class GStream:
    def __init__(self, g):
        self.g = iter(g)
        self.done = False

    def step(self):
        if self.done:
            return False
        try:
            next(self.g)
            return True
        except StopIteration:
            self.done = True
            return False

    def finish(self):
        while self.step():
            pass


def _weave_until(master, cs, w_master=1, w_c=1):
    """Interleave master with the persistent queue `cs` (GStream list).
    Returns when master is exhausted; cs keeps its remaining state."""
    m = iter(master)
    while True:
        for _ in range(w_master):
            try:
                next(m)
            except StopIteration:
                return
        budget = w_c
        while budget > 0 and cs:
            if cs[0].step():
                budget -= 1
            else:
                cs.pop(0)


def _delay(n, g):
    for _ in range(n):
        yield
    yield from g


def _emit(ctx, tc, nc, aps, wp, vp, bpk, flags):
    wpool = ctx.enter_context(tc.tile_pool(name="weights", bufs=1))
    wr = wpool.tile([128, wp.n], f32r, name="wr", tag="wr")
    vec = wpool.tile([128, vp.n], f32, name="vec", tag="vec")
    nc.sync.dma_start(vec[:], aps["vpack"][:])
    bw = wpool.tile([128, bpk.n], bf16, name="bw", tag="bw")
    nc.sync.dma_start(bw[:], aps["bpack"][:])
    with tc.tile_pool(name="wstage", bufs=1) as stpool:
        wstage = stpool.tile([128, wp.n], f32, name="wstage")
        nc.sync.dma_start(wstage[:], aps["wpack"][:])
        for o in range(0, wp.n, 8192):
            e = min(wp.n, o + 8192)
            nc.vector.tensor_copy(wr[:, o:e], wstage[:, o:e])

    def WR(name):
        o, c = wp.off[name]
        return wr[:, o:o + c]

    def VP(name):
        o, c = vp.off[name]
        return vec[:, o:o + c]

    def BR(name):
        o, c = bpk.off[name]
        return bw[:, o:o + c]

    def mm(psum_ap, lhsT_ap, rhs_ap, start, stop, kp=128):
        nc.tensor.matmul(psum_ap, lhsT_ap[:kp, :], rhs_ap[:kp, :],
                         start=start, stop=stop)

    def dAW(mi, cc, g):
        return BR(f"dA{g}" if flags["A_shared"] else f"dA{mi}{cc}{g}")

    I64 = VP("I64")
    I128 = VP("I128")

    # ---- long-lived SBUF state --------------------------------------
    blk = ctx.enter_context(tc.tile_pool(name="blk", bufs=1))
    # s tiles: n/l halves need full (128, CBLK); mix writes them in place
    # over the ym values (same storage). t halves use a small ring.
    s_full = {nm: blk.tile([128, CBLK], bf16, name=f"s_{nm}",
                           tag=f"s_{nm[0]}")
              for nm in ("n0", "n1", "l0", "l1")}  # tag per family: 2 tags
    statp = ctx.enter_context(tc.tile_pool(name="stat", bufs=1))
    stat, rnm = {}, {}
    for bname, _, _, _, _ in HBLOCKS:
        for ph in ("a", "f"):
            stat[(bname, ph)] = statp.tile(
                [128, 2 * NW], f32, name=f"st_{bname}{ph}",
                tag=f"st_{bname}{ph}")
            rnm[(bname, ph)] = statp.tile(
                [128, 2 * NW], bf16, name=f"rn_{bname}{ph}",
                tag=f"rn_{bname}{ph}")

    # ---- shared PSUM tags (8 banks total) ---------------------------
    ps = ctx.enter_context(tc.tile_pool(name="ps", bufs=1, space="PSUM"))

    def psA(name):
        return ps.tile([128, 2 * CT], f32, name=name, tag="psA", bufs=2)

    def psS(name):
        return ps.tile([1, 2 * CT], f32, name=name, tag="psA", bufs=2)

    def psY(name):
        return ps.tile([128, CT], f32, name=name, tag="psY", bufs=2)

    def psC(name):
        return ps.tile([128, CT], f32, name=name, tag="psC", bufs=2)

    rg = ctx.enter_context(tc.tile_pool(name="rg", bufs=1))

    # ---- LN helpers -------------------------------------------------
    def ln_stats(src_ap, sq_ap, statT, c0):
        ob = BR("onesDb")[:, 0:1]
        of = WR("onesD")[:, 0:1]
        pmq = psS("pmq")
        mm(pmq[:, 0:CT], ob if src_ap.dtype == bf16 else of, src_ap,
           True, True)
        mm(pmq[:, CT:2 * CT], ob if sq_ap.dtype == bf16 else of, sq_ap,
           True, True)
        sst = rg.tile([1, 2 * CT], f32, name="sst", tag="sst", bufs=1)
        nc.scalar.activation(sst[:], pmq[:], AF.Copy)
        p0 = (c0 // CT) * 16
        nc.sync.dma_start(
            stat[statT][p0:p0 + 16, :].rearrange("p (h w) -> h p w", h=2),
            sst[:].rearrange("x (h p w) -> x h p w", h=2, p=16))

    def ln_finish(statT, c0):
        """Finish LN scale factors for one column tile (16 partitions)."""
        p0 = (c0 // CT) * 16
        sT = stat[statT]
        rT = rnm[statT]
        m_t = sT[p0:p0 + 16, 0:NW]
        q_t = sT[p0:p0 + 16, NW:2 * NW]
        var = rg.tile([16, NW], f32, name="var", tag="lnvar", bufs=2)
        nc.vector.tensor_mul(var[:], m_t, m_t)
        nc.vector.tensor_sub(var[:], q_t, var[:])
        lnv = rg.tile([16, NW], f32, name="lnv", tag="lnlnv", bufs=2)
        nc.scalar.activation(lnv[:], var[:], AF.Ln, bias=VP("eps")[0:16])
        r_t = rg.tile([16, NW], f32, name="lnr", tag="lnr", bufs=2)
        nc.scalar.activation(r_t[:], lnv[:], AF.Exp, scale=-0.5)
        nmr = rg.tile([16, NW], f32, name="nmr", tag="lnnmr", bufs=2)
        nc.vector.tensor_mul(nmr[:], m_t, r_t[:])
        nc.vector.tensor_scalar(nmr[:], nmr[:], -1.0, None, OP.mult)
        nc.vector.tensor_copy(rT[p0:p0 + 16, 0:NW], r_t[:])
        nc.vector.tensor_copy(rT[p0:p0 + 16, NW:2 * NW], nmr[:])
        bn, ph = statT
        nc.sync.dma_start(
            aps[f"scr_rn_{bn}_{ph}"][:, p0:p0 + 16, :],
            rT[p0:p0 + 16, :].rearrange("p (h w) -> h p w", h=2))

    def ln_apply(src_ap, rnmT, c0, gk, bk, out_ap):
        p0 = (c0 // CT) * 16
        rnm2 = rg.tile([128, 2 * CT], bf16, name="rnm2", tag="rnm2",
                       bufs=2)
        bn, ph = rnmT
        nc.sync.dma_start(
            rnm2[:].rearrange("j (h ab) -> j h ab", h=2),
            aps[f"scr_rn_{bn}_{ph}"][:, p0:p0 + 16, :]
            .rearrange("h p w -> h (p w)")
            .unsqueeze(0).broadcast_to((128, 2, CT)))
        t1 = rg.tile([128, CT], bf16, name="t1", tag="t1", bufs=2)
        nc.vector.tensor_mul(t1[:], src_ap, rnm2[:, 0:CT])
        if gk is None:
            nc.vector.tensor_add(out_ap, t1[:], rnm2[:, CT:2 * CT])
        else:
            nc.vector.tensor_add(t1[:], t1[:], rnm2[:, CT:2 * CT])
            nc.vector.tensor_scalar(out_ap, t1[:], VP(gk), VP(bk),
                                    OP.mult, OP.add)

    tiles = {}

    # ================= phase A1 (8 units) ============================
    def g_a1(bname, sqp):
        _, mi, xkey, q_off, _ = HB[bname]
        xcb = [sqp.tile([128, CBLK], bf16, name=f"xcb{cc}", tag=f"xcb{cc}")
               for cc in range(2)]
        zsb = [sqp.tile([128, CBLK], bf16, name=f"zsb{cc}", tag=f"zsb{cc}")
               for cc in range(2)]
        xT = sqp.tile([128, CBLK], bf16, name="xT", tag="xT")
        tiles[bname] = (xcb, zsb, xT)
        for c0 in range(0, CBLK, CT):
            q0 = c0 // W
            raw = rg.tile([64, 8 * 128], f32, name="raw", tag="raw", bufs=2)
            nc.sync.dma_start(raw[:],
                              aps[xkey][:, q_off + q0:q_off + q0 + 8, :])
            pt = psC("pt")
            for i in range(8):
                nc.tensor.transpose(pt[:, i * 64:(i + 1) * 64],
                                    raw[:, i * 128:(i + 1) * 128],
                                    I64[:64, :64])
            nc.scalar.activation(xT[:, c0:c0 + CT], pt[:], AF.Copy)
            xt_t = xT[:, c0:c0 + CT]
            pxc2 = psA("pxc2")
            for cc in range(2):
                mm(pxc2[:, cc * CT:(cc + 1) * CT],
                   BR(f"win{mi}")[:, cc * 128:(cc + 1) * 128], xt_t,
                   True, True)
            pz2 = psA("pz2")
            for cc in range(2):
                mm(pz2[:, cc * CT:(cc + 1) * CT],
                   BR(f"win{mi}")[:, (2 + cc) * 128:(3 + cc) * 128],
                   xt_t, True, True)
            for cc in range(2):
                nc.scalar.activation(zsb[cc][:, c0:c0 + CT],
                                     pz2[:, cc * CT:(cc + 1) * CT],
                                     AF.Silu)
            zc = rg.tile([128, 2 * CT], bf16, name="zc", tag="zc", bufs=1)
            nc.scalar.activation(zc[:], pxc2[:], AF.Copy)
            acc2 = rg.tile([128, 2 * CT], f32, name="acc2", tag="acc2",
                           bufs=1)
            for cc in range(2):
                pzv = zc[:, cc * CT:(cc + 1) * CT]
                accv = acc2[:, cc * CT:(cc + 1) * CT]
                cw = VP(f"cw{mi}{cc}")
                srcr = pzv.rearrange("p (q t) -> p q t", t=W)
                accr = accv.rearrange("p (q t) -> p q t", t=W)
                nc.vector.tensor_scalar(accv, pzv, cw[:, 3:4], None,
                                        OP.mult)
                for k in range(3):
                    sh = 3 - k
                    nc.vector.scalar_tensor_tensor(
                        accr[:, :, sh:W], srcr[:, :, 0:W - sh],
                        cw[:, k:k + 1], accr[:, :, sh:W],
                        OP.mult, OP.add)
            for cc in range(2):
                bias = (None if flags["convb_zero"]
                        else VP(f"cb{mi}{cc}"))
                if bias is None:
                    nc.scalar.activation(xcb[cc][:, c0:c0 + CT],
                                         acc2[:, cc * CT:(cc + 1) * CT],
                                         AF.Silu)
                else:
                    nc.scalar.activation(xcb[cc][:, c0:c0 + CT],
                                         acc2[:, cc * CT:(cc + 1) * CT],
                                         AF.Silu, bias=bias)
            pbc = psC("pbc")
            for cc in range(2):
                mm(pbc[:40, :], BR(f"bc{mi}{cc}"),
                   xcb[cc][:, c0:c0 + CT], cc == 0, cc == 1)
            bcs = rg.tile([40, CT], bf16, name="bcs", tag="bcs", bufs=2)
            nc.scalar.activation(bcs[:], pbc[:40, :], AF.Copy)
            nc.sync.dma_start(aps[f"scr_bc_{bname}"][:, c0:c0 + CT],
                              bcs[:])
            yield

    # ============ dt factory: one tile -> dts ring slot ==============
    def emit_factory(bname, mi, ci, dts_ring):
        xcb, _, _ = tiles[bname]
        c0 = ci * CT
        dtin = rg.tile([8, CT], bf16, name="dtin", tag="dtin", bufs=2)
        nc.sync.dma_start(dtin[:],
                          aps[f"scr_bc_{bname}"][0:8, c0:c0 + CT])
        pd = psA("pd")
        for cc in range(2):
            mm(pd[:, cc * CT:(cc + 1) * CT], BR(f"dtw{mi}{cc}"),
               dtin[:], True, True, kp=8)
        ez = rg.tile([128, 2 * CT], bf16, name="ez", tag="ez", bufs=1)
        if flags["dtb_const"] is not None:
            nc.scalar.activation(ez[:], pd[:], AF.Exp, bias=VP("dtbc"))
        else:
            for cc in range(2):
                nc.scalar.activation(
                    ez[:, cc * CT:(cc + 1) * CT],
                    pd[:, cc * CT:(cc + 1) * CT],
                    AF.Exp, bias=VP(f"dtb{mi}{cc}"))
        dts = rg.tile([128, 2 * CT], bf16, name="dts", tag="dts", bufs=3)
        nc.scalar.activation(dts[:], ez[:], AF.Ln, bias=1.0)
        dtx = rg.tile([128, 2 * CT], bf16, name="dtx", tag="dtx", bufs=2)
        for cc in range(2):
            nc.vector.tensor_mul(dtx[:, cc * CT:(cc + 1) * CT],
                                 dts[:, cc * CT:(cc + 1) * CT],
                                 xcb[cc][:, c0:c0 + CT])
        for cc in range(2):
            nc.sync.dma_start(
                aps[f"scr_dtx_{bname}"][cc, ci]
                .rearrange("p (g c) -> g p c", g=16),
                dtx[:, cc * CT:(cc + 1) * CT])
        dtv = dts[:].rearrange("p (x t) -> p x t", t=W)
        nc.vector.tensor_scalar(dtv[:, :, 0:1], dtv[:, :, 0:1],
                                0.0, 1.0e4, OP.mult, OP.add)
        dts_ring[ci] = dts

    # ============ scan unit (one tile) ===============================
    def emit_scan(bname, mi, ci, dts_ring, is_t, s_tile, s_c0):
        xcb, zsb, xT = tiles[bname]
        c0 = ci * CT
        dts = dts_ring.pop(ci)
        Brep = rg.tile([128, CT], bf16, name="Brep", tag="Brep", bufs=2)
        nc.sync.dma_start(
            Brep[:],
            aps[f"scr_bc_{bname}"][8:24, c0:c0 + CT]
            .unsqueeze(0).broadcast_to((8, 16, CT)))
        Crep = rg.tile([128, CT], bf16, name="Crep", tag="Crep", bufs=2)
        nc.sync.dma_start(
            Crep[:],
            aps[f"scr_bc_{bname}"][24:40, c0:c0 + CT]
            .unsqueeze(0).broadcast_to((8, 16, CT)))
        ues = []
        for cc in range(2):
            halves = []
            for hf in range(2):
                ueh = rg.tile([128, 8 * CT], bf16, name="ueh",
                              tag=f"ueh{hf}", bufs=2 - hf)
                nc.sync.dma_start(
                    ueh[:],
                    aps[f"scr_dtx_{bname}"]
                    [cc, ci, :, hf * 8 * CT:(hf + 1) * 8 * CT]
                    .unsqueeze(1).broadcast_to((8, 16, 8 * CT)))
                halves.append(ueh)
            ues.append(halves)
        Brep_b = Brep[:].unsqueeze(1).broadcast_to((128, 2, CT))
        Crep_b = Crep[:].unsqueeze(1).broadcast_to((128, 2, CT))
        gzs = []
        for cc in range(2):
            pY = psY(f"pY{cc}")
            stage = []
            for gp in range(11):
                if gp < 8:
                    pP = psA("pP")
                    for i in range(2):
                        g = gp * 2 + i
                        mm(pP[:, i * CT:(i + 1) * CT],
                           dAW(mi, cc, g),
                           dts[:, cc * CT:(cc + 1) * CT],
                           True, True)
                    dA = rg.tile([128, 2 * CT], bf16, name="dA",
                                 tag="dA", bufs=3)
                    nc.scalar.activation(dA[:], pP[:], AF.Exp)
                    u2 = rg.tile([128, 2 * CT], bf16, name="u2",
                                 tag="u2", bufs=3)
                    ueh = ues[cc][gp // 4]
                    sl = (gp % 4) * 2 * CT
                    eng = nc.gpsimd if gp % 4 == 3 else nc.vector
                    eng.tensor_mul(
                        u2[:].rearrange("p (i c) -> p i c", i=2),
                        ueh[:, sl:sl + 2 * CT]
                        .rearrange("p (i c) -> p i c", i=2),
                        Brep_b)
                    stage.append((gp, dA, u2))
                if gp >= 3:
                    gq, dAq, u2q = stage.pop(0)
                    h2 = rg.tile([128, 2 * CT], bf16, name="h2",
                                 tag="h2", bufs=2)
                    for i in range(2):
                        nc.vector.tensor_tensor_scan(
                            h2[:, i * CT:(i + 1) * CT],
                            dAq[:, i * CT:(i + 1) * CT],
                            u2q[:, i * CT:(i + 1) * CT],
                            0.0, OP.mult, OP.add)
                    yh2 = rg.tile([128, 2 * CT], bf16, name="yh2",
                                  tag="yh2", bufs=2)
                    eng = nc.gpsimd if gq % 2 == 1 else nc.vector
                    eng.tensor_mul(
                        yh2[:].rearrange("p (i c) -> p i c", i=2),
                        h2[:].rearrange("p (i c) -> p i c", i=2),
                        Crep_b)
                    for i in range(2):
                        g = gq * 2 + i
                        mm(pY[:], BR(f"sum{g}"),
                           yh2[:, i * CT:(i + 1) * CT],
                           g == 0, g == 15)
            yg = rg.tile([128, CT], bf16, name=f"yg{cc}", tag=f"yg{cc}",
                         bufs=1)
            if flags["D_ones"]:
                nc.vector.tensor_add(yg[:], pY[:],
                                     xcb[cc][:, c0:c0 + CT])
            else:
                nc.vector.scalar_tensor_tensor(
                    yg[:], xcb[cc][:, c0:c0 + CT],
                    VP(f"Dp{mi}{cc}"), pY[:], OP.mult, OP.add)
            gz = rg.tile([128, CT], bf16, name=f"gz{cc}", tag=f"gz{cc}",
                         bufs=1)
            nc.gpsimd.tensor_mul(gz[:], yg[:], zsb[cc][:, c0:c0 + CT])
            gzs.append(gz)
        po = psC("po")
        for cc in range(2):
            mm(po[:], BR(f"wout{mi}{cc}"), gzs[cc][:], cc == 0, cc == 1)
        res = s_tile[:, s_c0:s_c0 + CT]
        if is_t:
            nc.vector.tensor_add(res, po[:], xT[:, c0:c0 + CT])
            s2 = rg.tile([128, CT], bf16, name="s2", tag="s2", bufs=1)
            nc.gpsimd.tensor_mul(s2[:], res, res)
            ln_stats(res, s2[:], (bname, "a"), c0)
            ln_finish((bname, "a"), c0)
        else:
            # n/l: write pre-mix value (ym) into the s tile; mix rewrites
            nc.scalar.activation(res, po[:], AF.Copy)

    # ========= factory + scan generator =====================
    # units: fac(0), fac(1), [scan(0), fac(2)], [scan(1), fac(3)], ...
    def g_fs(bname):
        _, mi, _, _, _ = HB[bname]
        is_t = bname[0] == "t"
        s_tile = s_full.get(bname)
        dts_ring = {}
        emit_factory(bname, mi, 0, dts_ring)
        yield
        emit_factory(bname, mi, 1, dts_ring)
        yield
        for ci in range(NT):
            if s_tile is not None:
                st, sc0 = s_tile, ci * CT
            else:
                st = rg.tile([128, CT], bf16, name="sT", tag="sT", bufs=8)
                tiles_s[(bname, ci)] = st
                sc0 = 0
            emit_scan(bname, mi, ci, dts_ring, is_t, st, sc0)
            if ci + 2 < NT:
                emit_factory(bname, mi, ci + 2, dts_ring)
            yield

    tiles_s = {}

    # ================= mix generator (8 units) =======================
    def g_mix(pair):
        nb, lb = pair
        q_n = HB[nb][3]
        q_l = HB[lb][3]
        for c0 in range(0, CBLK, CT):
            cat = {"n": s_full[nb][:, c0:c0 + CT],
                   "l": s_full[lb][:, c0:c0 + CT]}
            # both mix matmuls first: they read cat slices that the res
            # writes below overwrite in place (s tile doubles as ym)
            mss = []
            for mc in range(2):
                pmx = psC("pmx")
                for kc, kk in enumerate(("n", "l")):
                    mm(pmx[:], BR(f"mix{kc}{mc}"), cat[kk], kc == 0,
                       kc == 1)
                ms = rg.tile([128, CT], bf16, name="ms", tag=f"ms{mc}",
                             bufs=1)
                if flags["mixb_zero"]:
                    nc.scalar.activation(ms[:], pmx[:], AF.Silu)
                else:
                    nc.scalar.activation(ms[:], pmx[:], AF.Silu,
                                         bias=VP(f"mixb{mc}"))
                mss.append(ms)
            # t2 adds also read cat before any res write
            t2s = []
            for mc, key in enumerate(("n", "l")):
                t2 = rg.tile([128, CT], bf16, name="t2", tag=f"t2{mc}",
                             bufs=1)
                nc.vector.tensor_add(t2[:], cat[key], mss[mc][:])
                t2s.append(t2)
            for mc, (key, bn, xk, qo) in enumerate(
                    (("n", nb, "x_n", q_n), ("l", lb, "x_l", q_l))):
                q0 = qo + c0 // W
                raw = rg.tile([64, 8 * 128], f32, name="rawm", tag="raw",
                              bufs=2)
                nc.sync.dma_start(raw[:], aps[xk][:, q0:q0 + 8, :])
                ptx = psC("ptx")
                for i in range(8):
                    nc.tensor.transpose(ptx[:, i * 64:(i + 1) * 64],
                                        raw[:, i * 128:(i + 1) * 128],
                                        I64[:64, :64])
                res = s_full[bn][:, c0:c0 + CT]
                nc.vector.tensor_add(res, t2s[mc][:], ptx[:])
                s2m = rg.tile([128, CT], bf16, name="s2m", tag="s2m",
                              bufs=2)
                nc.gpsimd.tensor_mul(s2m[:], res, res)
                ln_stats(res, s2m[:], (bn, "a"), c0)
            yield

    # ================= phase C generator (16 units) ==================
    def g_C(bname, fin_a):
        _, mi, _, _, j0 = HB[bname]
        is_t = bname[0] == "t"
        for ci in range(NT):
            c0 = ci * CT
            if is_t:
                while (bname, ci) not in tiles_s:
                    yield   # scan hasn't produced this tile yet; spin
                src = tiles_s.pop((bname, ci))[:]
            else:
                src = s_full[bname][:, c0:c0 + CT]
            if fin_a:
                ln_finish((bname, "a"), c0)
            n1 = rg.tile([128, CT], bf16, name="n1", tag="n1", bufs=2)
            ga, gb = (None, None) if flags["an_id"] else \
                (f"ang{mi}", f"anb{mi}")
            ln_apply(src, (bname, "a"), c0, ga, gb, n1[:])
            hh = rg.tile([128, 4 * CT], bf16, name="hh", tag="hh", bufs=1)
            for hp in range(2):
                pf = psA("pf")
                for ci2 in range(2):
                    c4 = hp * 2 + ci2
                    mm(pf[:, ci2 * CT:(ci2 + 1) * CT],
                       BR(f"ff1{mi}")[:, c4 * 128:(c4 + 1) * 128],
                       n1[:], True, True)
                if flags["f1b_zero"]:
                    nc.scalar.activation(
                        hh[:, hp * 2 * CT:(hp + 1) * 2 * CT], pf[:],
                        AF.Lrelu, alpha=0.01)
                else:
                    for ci2 in range(2):
                        c4 = hp * 2 + ci2
                        nc.scalar.activation(
                            hh[:, c4 * CT:(c4 + 1) * CT],
                            pf[:, ci2 * CT:(ci2 + 1) * CT], AF.Lrelu,
                            bias=VP(f"f1b{mi}{c4}"), alpha=0.01)
            pf2 = psC("pf2")
            for c4 in range(4):
                mm(pf2[:], BR(f"ff2{mi}{c4}"),
                   hh[:, c4 * CT:(c4 + 1) * CT], c4 == 0, c4 == 3)
            sf = rg.tile([128, CT], bf16, name="sf", tag="sf", bufs=2)
            if flags["f2b_zero"]:
                nc.vector.tensor_add(sf[:], pf2[:], n1[:])
            else:
                nc.vector.scalar_tensor_tensor(sf[:], pf2[:],
                                               VP(f"f2b{mi}"),
                                               n1[:], OP.add, OP.add)
            s2f = rg.tile([128, CT], bf16, name="s2f", tag="s2f", bufs=1)
            nc.gpsimd.tensor_mul(s2f[:], sf[:], sf[:])
            ln_stats(sf[:], s2f[:], (bname, "f"), c0)
            yield
            ln_finish((bname, "f"), c0)
            ga, gb = (None, None) if flags["fln_id"] else \
                (f"flg{mi}", f"flb{mi}")
            n2 = rg.tile([128, CT], f32, name="n2", tag="n2", bufs=1)
            ln_apply(sf[:], (bname, "f"), c0, ga, gb, n2[:])
            pto = psC("pto")
            for c in range(4):
                nc.tensor.transpose(pto[:, c * 128:(c + 1) * 128],
                                    n2[:, c * 128:(c + 1) * 128],
                                    I128)
            ot = rg.tile([128, CT], f32, name="ot", tag="ot", bufs=1)
            nc.vector.tensor_copy(ot[:], pto[:])
            q0 = c0 // W
            for qh in range(2):
                nc.sync.dma_start(
                    aps["out"][:, j0 + q0:j0 + q0 + 8, :]
                    .rearrange("t (c q) d -> q t c d", c=4)[qh],
                    ot[qh * 64:(qh + 1) * 64, :]
                    .rearrange("t (c d) -> t c d", c=4))
            yield

    # ===================== master schedule ===========================
    with tc.tile_pool(name="a_sq", bufs=1) as a_sqp:
        cq = []   # persistent queue of pending C generators

        def A1(bn):
            return g_a1(bn, a_sqp)

        # order: n0, l0, t0, n1, l1, t1, t2, t3 — a family's C phase fully
        # drains inside t-block windows before its s/sT tag is reused.
        _drain(A1("n0"))
        _drain(g_fs("n0"))
        _drain(A1("l0"))
        _drain(g_fs("l0"))
        _weave([(A1("t0"), 1), (g_mix(("n0", "l0")), 1)])
        C_n0 = GStream(g_C("n0", True))
        C_l0 = GStream(g_C("l0", True))
        C_t0 = GStream(_delay(5, g_C("t0", False)))
        cq.extend([C_n0, C_l0, C_t0])
        _weave_until(g_fs("t0"), cq, w_master=1, w_c=3)
        C_n0.finish()          # s_n tag is rewritten by scan(n1)
        _weave_until(A1("n1"), cq, w_master=1, w_c=2)
        _weave_until(g_fs("n1"), cq, w_master=1, w_c=2)
        C_l0.finish()          # s_l tag is rewritten by scan(l1)
        _weave_until(A1("l1"), cq, w_master=1, w_c=2)
        _weave_until(g_fs("l1"), cq, w_master=1, w_c=2)
        _weave([(A1("t1"), 1), (g_mix(("n1", "l1")), 1)])
        C_prev = C_t0
        cq.extend([GStream(g_C("n1", True)), GStream(g_C("l1", True))])
        for bn, nxt in (("t1", "t2"), ("t2", "t3"), ("t3", None)):
            C_prev.finish()    # sT ring slots reused by scan(bn)
            C_cur = GStream(_delay(5, g_C(bn, False)))
            cq.append(C_cur)
            _weave_until(g_fs(bn), cq, w_master=1, w_c=3)
            if nxt is not None:
                _weave_until(A1(nxt), cq, w_master=1, w_c=2)
            C_prev = C_cur
        for g in cq:
            g.finish()


def _build_program(wp, vp, bpk, flags):
    nc = bacc.Bacc("TRN2", target_bir_lowering=False, debug=False,
                   num_devices=N_CORES)
    aps = {}
    aps["x_n"] = nc.dram_tensor("x_n", [W, N, D], f32,
                                kind="ExternalInput").ap()
    aps["x_t"] = nc.dram_tensor("x_t", [W, E, D], f32,
                                kind="ExternalInput").ap()
    aps["x_l"] = nc.dram_tensor("x_l", [W, N, D], f32,
                                kind="ExternalInput").ap()
    aps["wpack"] = nc.dram_tensor("wpack", [128, wp.n], f32,
                                  kind="ExternalInput").ap()
    aps["vpack"] = nc.dram_tensor("vpack", [128, vp.n], f32,
                                  kind="ExternalInput").ap()
    aps["bpack"] = nc.dram_tensor("bpack", [128, bpk.n], bf16,
                                  kind="ExternalInput").ap()
    aps["out"] = nc.dram_tensor("out", [W, 2 * N + E, D], f32,
                                kind="ExternalOutput").ap()
    for bname, _, _, _, _ in HBLOCKS:
        aps[f"scr_bc_{bname}"] = nc.dram_tensor(
            f"scr_bc_{bname}", [40, CBLK], bf16).ap()
        aps[f"scr_dtx_{bname}"] = nc.dram_tensor(
            f"scr_dtx_{bname}", [2, NT, 8, 16 * CT], bf16).ap()
        for ph in ("a", "f"):
            aps[f"scr_rn_{bname}_{ph}"] = nc.dram_tensor(
                f"scr_rn_{bname}_{ph}", [2, 128, NW], bf16).ap()

    with tile.TileContext(nc) as tc:
        with ExitStack() as ctx:
            _emit(ctx, tc, nc, aps, wp, vp, bpk, flags)
    nc.compile()
    return nc


_CACHE = {}


def kernel(**inputs):
    wp, vp, bpk, flags = _host_pack(inputs)
    if "prog" not in _CACHE:
        _CACHE["prog"] = _build_program(wp, vp, bpk, flags)
    nc = _CACHE["prog"]
    wpack, vpack = wp.build(), vp.build()
    bpack = bpk.build().astype(ml_dtypes.bfloat16)
    in_maps = []
    for b in range(B):
        in_maps.append({
            "x_n": np.ascontiguousarray(inputs["x_node"][b]),
            "x_t": np.ascontiguousarray(inputs["x_trace"][b]),
            "x_l": np.ascontiguousarray(inputs["x_log"][b]),
            "wpack": wpack,
            "vpack": vpack,
            "bpack": bpack,
        })
    res = run_bass_kernel_spmd(nc, in_maps, list(range(N_CORES)))
    out = np.stack([res.results[b]["out"] for b in range(B)], axis=0)
    return out.astype(np.float32)


# revision 22
# speedup vs baseline: 1.0840x; 1.0840x over previous
"""Trainium2 Bass kernel for nn_Encoder (tri-modal Mamba encoder), v3.

kernel(**inputs) takes FULL unsharded numpy inputs and returns the FULL
output (B, W, 2N+E, D). Batch B=8 is sharded across 8 NeuronCores (pure
data parallel, no collectives); params are replicated.

v3 vs v2 (same math, new orchestration):
- Half-block (64-seq) processing units, software-pipelined end to end:
  the AddNorm+FFN+output phase (C) of each half-block is emitted as a
  persistent generator that drains into whatever later window has engine
  slack, so the serial phase-C tail is gone.
- LayerNorm stats finish per column-tile with rsqrt = exp(-0.5*ln(v+eps))
  so interleaved phases stay inside the natural_log_exp activation-table
  set (Silu windows are kept separate: A1 and mix).
- s / LN stats / LN scale factors live in SBUF; ym merged into s tiles.
- One shared 8-bank PSUM tag discipline (psA 2x[128,1024]f32,
  psY 2x[128,512], psC 2x[128,512]) across all phases.
- Output path: 4 batched f32 transposes per 512-col tile, one copy, one
  DMA (q t d scatter) straight from the LN-f apply.
"""

import ml_dtypes
import numpy as np
from contextlib import ExitStack

import concourse.bass as bass
import concourse.tile as tile
from concourse import bacc, mybir
from concourse.bass_utils import run_bass_kernel_spmd

D, DI, SS, KK, RR = 128, 256, 16, 4, 8
B, W, N, E = 8, 64, 128, 256
Q = 64                       # seqs per half-block
CBLK = Q * W                 # 4096 cols per half-block
CT = 512                     # column tile (8 seqs)
NT = CBLK // CT              # 8 tiles per half-block
NW = CBLK // 128             # 32 stat cols per partition
f32 = mybir.dt.float32
f32r = mybir.dt.float32r
bf16 = mybir.dt.bfloat16
AF = mybir.ActivationFunctionType
OP = mybir.AluOpType

# (name, modality, input key, q offset, output entity offset)
HBLOCKS = [("n0", 0, "x_n", 0, 0), ("n1", 0, "x_n", 64, 64),
           ("l0", 2, "x_l", 0, 384), ("l1", 2, "x_l", 64, 448),
           ("t0", 1, "x_t", 0, 128), ("t1", 1, "x_t", 64, 192),
           ("t2", 1, "x_t", 128, 256), ("t3", 1, "x_t", 192, 320)]
HB = {b[0]: b for b in HBLOCKS}
N_CORES = 8
LN_EPS = 1e-5


class Pack:
    def __init__(self):
        self.cols = []
        self.off = {}
        self.n = 0

    def add(self, name, arr, dtype=np.float32):
        arr = np.asarray(arr, dtype)
        assert arr.ndim == 2 and arr.shape[0] <= 128
        a = np.zeros((128, arr.shape[1]), dtype)
        a[: arr.shape[0]] = arr
        self.off[name] = (self.n, arr.shape[1])
        self.cols.append(a)
        self.n += arr.shape[1]

    def build(self):
        return np.concatenate(self.cols, axis=1)


def _host_pack(inp):
    """Returns (wp f32-staged-to-f32r, vp f32, bp bf16, flags)."""
    flags = {}
    dtb = np.asarray(inp["mp_dt_b"], np.float64)
    flags["dtb_const"] = float(dtb.flat[0]) if np.ptp(dtb) < 1e-12 else None
    flags["D_ones"] = bool(np.allclose(np.asarray(inp["mp_D"]), 1.0))
    flags["convb_zero"] = bool(np.all(np.asarray(inp["mp_conv_b"]) == 0.0))
    flags["f1b_zero"] = bool(np.all(np.asarray(inp["ff1_b"]) == 0.0))
    flags["f2b_zero"] = bool(np.all(np.asarray(inp["ff2_b"]) == 0.0))
    flags["an_id"] = bool(np.all(np.asarray(inp["an_g"]) == 1.0)
                          and np.all(np.asarray(inp["an_b"]) == 0.0))
    flags["fln_id"] = bool(np.all(np.asarray(inp["fln_g"]) == 1.0)
                           and np.all(np.asarray(inp["fln_b"]) == 0.0))
    flags["mixb_zero"] = bool(np.all(np.asarray(inp["mix_b"]) == 0.0))
    A = -np.exp(np.asarray(inp["mp_Alog"], np.float64))      # (3, DI, S)
    flags["A_shared"] = bool(
        np.ptp(A, axis=(0, 1)).max() < 1e-9 * np.abs(A).max())

    bp = Pack()   # bf16 weights
    for g in range(16):
        sm = np.zeros((128, 128), np.float32)
        for k in range(128):
            sm[k, g * 8 + k // 16] = 1.0
        bp.add(f"sum{g}", sm)

    def delta_A(Am, cc, g):
        dl = np.zeros((128, 128), np.float32)
        for j in range(128):
            dl[g * 8 + j // 16, j] = Am[cc * 128 + g * 8 + j // 16, j % 16]
        return dl

    if flags["A_shared"]:
        for g in range(16):
            bp.add(f"dA{g}", delta_A(A[0], 0, g))
    else:
        for m in range(3):
            for cc in range(2):
                for g in range(16):
                    bp.add(f"dA{m}{cc}{g}", delta_A(A[m], cc, g))

    wp = Pack()   # fp32 staged -> f32r on device
    vp = Pack()   # fp32 per-partition vectors
    for m in range(3):
        bp.add(f"win{m}", inp["mp_in"][m])                   # (D, 512)
        wxp = inp["mp_xproj"][m]                             # (DI, 40)
        for cc in range(2):
            bp.add(f"bc{m}{cc}", wxp[cc * 128:(cc + 1) * 128])
        dtw = inp["mp_dt_w"][m]                              # (R, DI)
        for cc in range(2):
            bp.add(f"dtw{m}{cc}", dtw[:, cc * 128:(cc + 1) * 128])
        wout = inp["mp_out"][m]                              # (DI, D)
        for cc in range(2):
            bp.add(f"wout{m}{cc}", wout[cc * 128:(cc + 1) * 128])
        bp.add(f"ff1{m}", inp["ff1_w"][m])                   # (D, 512)
        ff2 = inp["ff2_w"][m]                                # (4D, D)
        for c4 in range(4):
            bp.add(f"ff2{m}{c4}", ff2[c4 * 128:(c4 + 1) * 128])
    mixw = inp["mix_w"]
    for kc in range(2):
        for mc in range(2):
            bp.add(f"mix{kc}{mc}", mixw[kc * 128:(kc + 1) * 128,
                                        mc * 128:(mc + 1) * 128])
    wp.add("onesD", np.full((128, 1), 1.0 / D, np.float32))
    bp.add("onesDb", np.full((128, 1), 1.0 / D, np.float32))
    wp.add("ones1", np.ones((1, 128), np.float32))

    vp.add("eps", np.full((128, 1), LN_EPS, np.float32))
    if flags["dtb_const"] is not None:
        vp.add("dtbc", np.full((128, 1), flags["dtb_const"], np.float32))
    vp.add("I64", np.eye(64, dtype=np.float32))
    vp.add("I128", np.eye(128, dtype=np.float32))
    for m in range(3):
        cw = inp["mp_conv_w"][m]
        for cc in range(2):
            sl = slice(cc * 128, (cc + 1) * 128)
            vp.add(f"cw{m}{cc}", cw[sl])                     # 4 cols
            if not flags["convb_zero"]:
                vp.add(f"cb{m}{cc}", inp["mp_conv_b"][m][sl, None])
            if flags["dtb_const"] is None:
                vp.add(f"dtb{m}{cc}", inp["mp_dt_b"][m][sl, None])
            if not flags["D_ones"]:
                vp.add(f"Dp{m}{cc}", inp["mp_D"][m][sl, None])
        if not flags["f1b_zero"]:
            for c4 in range(4):
                vp.add(f"f1b{m}{c4}",
                       inp["ff1_b"][m][c4 * 128:(c4 + 1) * 128, None])
        if not flags["f2b_zero"]:
            vp.add(f"f2b{m}", inp["ff2_b"][m][:, None])
        if not flags["an_id"]:
            vp.add(f"ang{m}", inp["an_g"][m][:, None])
            vp.add(f"anb{m}", inp["an_b"][m][:, None])
        if not flags["fln_id"]:
            vp.add(f"flg{m}", inp["fln_g"][m][:, None])
            vp.add(f"flb{m}", inp["fln_b"][m][:, None])
    if not flags["mixb_zero"]:
        for mc in range(2):
            vp.add(f"mixb{mc}", inp["mix_b"][mc * 128:(mc + 1) * 128, None])
    return wp, vp, bp, flags


def _drain(g):
    for _ in g:
        pass


def _weave(streams):
    """streams: (gen, weight) pairs; round-robin to exhaustion."""
    live = [[iter(g), w] for g, w in streams]
    while live:
        for ent in list(live):
            g, w = ent
            for _ in range(w):
                try:
                    next(g)
                except StopIteration:
                    live.remove(ent)
                    break


class GStream:
    def __init__(self, g):
        self.g = iter(g)
        self.done = False

    def step(self):
        if self.done:
            return False
        try:
            next(self.g)
            return True
        except StopIteration:
            self.done = True
            return False

    def finish(self):
        while self.step():
            pass


def _weave_until(master, cs, w_master=1, w_c=1):
    """Interleave master with the persistent queue `cs` (GStream list).
    Returns when master is exhausted; cs keeps its remaining state."""
    m = iter(master)
    while True:
        for _ in range(w_master):
            try:
                next(m)
            except StopIteration:
                return
        budget = w_c
        while budget > 0 and cs:
            if cs[0].step():
                budget -= 1
            else:
                cs.pop(0)


def _delay(n, g):
    for _ in range(n):
        yield
    yield from g


def _emit(ctx, tc, nc, aps, wp, vp, bpk, flags):
    wpool = ctx.enter_context(tc.tile_pool(name="weights", bufs=1))
    wr = wpool.tile([128, wp.n], f32r, name="wr", tag="wr")
    vec = wpool.tile([128, vp.n], f32, name="vec", tag="vec")
    nc.sync.dma_start(vec[:], aps["vpack"][:])
    bw = wpool.tile([128, bpk.n], bf16, name="bw", tag="bw")
    nc.sync.dma_start(bw[:], aps["bpack"][:])
    with tc.tile_pool(name="wstage", bufs=1) as stpool:
        wstage = stpool.tile([128, wp.n], f32, name="wstage")
        nc.sync.dma_start(wstage[:], aps["wpack"][:])
        for o in range(0, wp.n, 8192):
            e = min(wp.n, o + 8192)
            nc.vector.tensor_copy(wr[:, o:e], wstage[:, o:e])

    def WR(name):
        o, c = wp.off[name]
        return wr[:, o:o + c]

    def VP(name):
        o, c = vp.off[name]
        return vec[:, o:o + c]

    def BR(name):
        o, c = bpk.off[name]
        return bw[:, o:o + c]

    def mm(psum_ap, lhsT_ap, rhs_ap, start, stop, kp=128):
        nc.tensor.matmul(psum_ap, lhsT_ap[:kp, :], rhs_ap[:kp, :],
                         start=start, stop=stop)

    def dAW(mi, cc, g):
        return BR(f"dA{g}" if flags["A_shared"] else f"dA{mi}{cc}{g}")

    I64 = VP("I64")
    I128 = VP("I128")

    # ---- long-lived SBUF state --------------------------------------
    blk = ctx.enter_context(tc.tile_pool(name="blk", bufs=1))
    # s tiles: n/l halves need full (128, CBLK); mix writes them in place
    # over the ym values (same storage). t halves use a small ring.
    s_full = {nm: blk.tile([128, CBLK], bf16, name=f"s_{nm}",
                           tag=f"s_{nm[0]}")
              for nm in ("n0", "n1", "l0", "l1")}  # tag per family: 2 tags
    statp = ctx.enter_context(tc.tile_pool(name="stat", bufs=1))
    stat, rnm = {}, {}
    for bname, _, _, _, _ in HBLOCKS:
        for ph in ("a", "f"):
            stat[(bname, ph)] = statp.tile(
                [128, 2 * NW], f32, name=f"st_{bname}{ph}",
                tag=f"st_{bname}{ph}")
            rnm[(bname, ph)] = statp.tile(
                [128, 2 * NW], bf16, name=f"rn_{bname}{ph}",
                tag=f"rn_{bname}{ph}")

    # ---- shared PSUM tags (8 banks total) ---------------------------
    ps = ctx.enter_context(tc.tile_pool(name="ps", bufs=1, space="PSUM"))

    def psA(name):
        return ps.tile([128, 2 * CT], f32, name=name, tag="psA", bufs=2)

    def psS(name):
        return ps.tile([1, 2 * CT], f32, name=name, tag="psA", bufs=2)

    def psY(name):
        return ps.tile([128, CT], f32, name=name, tag="psY", bufs=2)

    def psC(name):
        return ps.tile([128, CT], f32, name=name, tag="psC", bufs=2)

    rg = ctx.enter_context(tc.tile_pool(name="rg", bufs=1))

    # ---- LN helpers -------------------------------------------------
    def ln_stats(src_ap, sq_ap, statT, c0):
        ob = BR("onesDb")[:, 0:1]
        of = WR("onesD")[:, 0:1]
        pmq = psS("pmq")
        mm(pmq[:, 0:CT], ob if src_ap.dtype == bf16 else of, src_ap,
           True, True)
        mm(pmq[:, CT:2 * CT], ob if sq_ap.dtype == bf16 else of, sq_ap,
           True, True)
        sst = rg.tile([1, 2 * CT], f32, name="sst", tag="sst", bufs=1)
        nc.scalar.activation(sst[:], pmq[:], AF.Copy)
        p0 = (c0 // CT) * 16
        nc.sync.dma_start(
            stat[statT][p0:p0 + 16, :].rearrange("p (h w) -> h p w", h=2),
            sst[:].rearrange("x (h p w) -> x h p w", h=2, p=16))

    def ln_finish(statT, c0):
        """Finish LN scale factors for one column tile (16 partitions)."""
        p0 = (c0 // CT) * 16
        sT = stat[statT]
        rT = rnm[statT]
        m_t = sT[p0:p0 + 16, 0:NW]
        q_t = sT[p0:p0 + 16, NW:2 * NW]
        var = rg.tile([16, NW], f32, name="var", tag="lnvar", bufs=2)
        nc.vector.tensor_mul(var[:], m_t, m_t)
        nc.vector.tensor_sub(var[:], q_t, var[:])
        lnv = rg.tile([16, NW], f32, name="lnv", tag="lnlnv", bufs=2)
        nc.scalar.activation(lnv[:], var[:], AF.Ln, bias=VP("eps")[0:16])
        r_t = rg.tile([16, NW], f32, name="lnr", tag="lnr", bufs=2)
        nc.scalar.activation(r_t[:], lnv[:], AF.Exp, scale=-0.5)
        nmr = rg.tile([16, NW], f32, name="nmr", tag="lnnmr", bufs=2)
        nc.vector.tensor_mul(nmr[:], m_t, r_t[:])
        nc.vector.tensor_scalar(nmr[:], nmr[:], -1.0, None, OP.mult)
        nc.vector.tensor_copy(rT[p0:p0 + 16, 0:NW], r_t[:])
        nc.vector.tensor_copy(rT[p0:p0 + 16, NW:2 * NW], nmr[:])
        bn, ph = statT
        nc.sync.dma_start(
            aps[f"scr_rn_{bn}_{ph}"][:, p0:p0 + 16, :],
            rT[p0:p0 + 16, :].rearrange("p (h w) -> h p w", h=2))

    def ln_apply(src_ap, rnmT, c0, gk, bk, out_ap):
        p0 = (c0 // CT) * 16
        rnm2 = rg.tile([128, 2 * CT], bf16, name="rnm2", tag="rnm2",
                       bufs=2)
        bn, ph = rnmT
        nc.sync.dma_start(
            rnm2[:].rearrange("j (h ab) -> j h ab", h=2),
            aps[f"scr_rn_{bn}_{ph}"][:, p0:p0 + 16, :]
            .rearrange("h p w -> h (p w)")
            .unsqueeze(0).broadcast_to((128, 2, CT)))
        t1 = rg.tile([128, CT], bf16, name="t1", tag="t1", bufs=2)
        nc.vector.tensor_mul(t1[:], src_ap, rnm2[:, 0:CT])
        if gk is None:
            nc.vector.tensor_add(out_ap, t1[:], rnm2[:, CT:2 * CT])
        else:
            nc.vector.tensor_add(t1[:], t1[:], rnm2[:, CT:2 * CT])
            nc.vector.tensor_scalar(out_ap, t1[:], VP(gk), VP(bk),
                                    OP.mult, OP.add)

    tiles = {}

    # ================= phase A1 (8 units) ============================
    def g_a1(bname, sqp):
        _, mi, xkey, q_off, _ = HB[bname]
        xcb = [sqp.tile([128, CBLK], bf16, name=f"xcb{cc}", tag=f"xcb{cc}")
               for cc in range(2)]
        zsb = [sqp.tile([128, CBLK], bf16, name=f"zsb{cc}", tag=f"zsb{cc}")
               for cc in range(2)]
        xT = sqp.tile([128, CBLK], bf16, name="xT", tag="xT")
        tiles[bname] = (xcb, zsb, xT)
        for c0 in range(0, CBLK, CT):
            q0 = c0 // W
            raw = rg.tile([64, 8 * 128], f32, name="raw", tag="raw", bufs=2)
            nc.sync.dma_start(raw[:],
                              aps[xkey][:, q_off + q0:q_off + q0 + 8, :])
            pt = psC("pt")
            for i in range(8):
                nc.tensor.transpose(pt[:, i * 64:(i + 1) * 64],
                                    raw[:, i * 128:(i + 1) * 128],
                                    I64[:64, :64])
            nc.scalar.activation(xT[:, c0:c0 + CT], pt[:], AF.Copy)
            xt_t = xT[:, c0:c0 + CT]
            pxc2 = psA("pxc2")
            for cc in range(2):
                mm(pxc2[:, cc * CT:(cc + 1) * CT],
                   BR(f"win{mi}")[:, cc * 128:(cc + 1) * 128], xt_t,
                   True, True)
            pz2 = psA("pz2")
            for cc in range(2):
                mm(pz2[:, cc * CT:(cc + 1) * CT],
                   BR(f"win{mi}")[:, (2 + cc) * 128:(3 + cc) * 128],
                   xt_t, True, True)
            for cc in range(2):
                nc.scalar.activation(zsb[cc][:, c0:c0 + CT],
                                     pz2[:, cc * CT:(cc + 1) * CT],
                                     AF.Silu)
            zc = rg.tile([128, 2 * CT], bf16, name="zc", tag="zc", bufs=1)
            nc.scalar.activation(zc[:], pxc2[:], AF.Copy)
            acc2 = rg.tile([128, 2 * CT], f32, name="acc2", tag="acc2",
                           bufs=1)
            for cc in range(2):
                pzv = zc[:, cc * CT:(cc + 1) * CT]
                accv = acc2[:, cc * CT:(cc + 1) * CT]
                cw = VP(f"cw{mi}{cc}")
                srcr = pzv.rearrange("p (q t) -> p q t", t=W)
                accr = accv.rearrange("p (q t) -> p q t", t=W)
                nc.vector.tensor_scalar(accv, pzv, cw[:, 3:4], None,
                                        OP.mult)
                for k in range(3):
                    sh = 3 - k
                    nc.vector.scalar_tensor_tensor(
                        accr[:, :, sh:W], srcr[:, :, 0:W - sh],
                        cw[:, k:k + 1], accr[:, :, sh:W],
                        OP.mult, OP.add)
            for cc in range(2):
                bias = (None if flags["convb_zero"]
                        else VP(f"cb{mi}{cc}"))
                if bias is None:
                    nc.scalar.activation(xcb[cc][:, c0:c0 + CT],
                                         acc2[:, cc * CT:(cc + 1) * CT],
                                         AF.Silu)
                else:
                    nc.scalar.activation(xcb[cc][:, c0:c0 + CT],
                                         acc2[:, cc * CT:(cc + 1) * CT],
                                         AF.Silu, bias=bias)
            pbc = psC("pbc")
            for cc in range(2):
                mm(pbc[:40, :], BR(f"bc{mi}{cc}"),
                   xcb[cc][:, c0:c0 + CT], cc == 0, cc == 1)
            bcs = rg.tile([40, CT], bf16, name="bcs", tag="bcs", bufs=2)
            nc.scalar.activation(bcs[:], pbc[:40, :], AF.Copy)
            nc.sync.dma_start(aps[f"scr_bc_{bname}"][:, c0:c0 + CT],
                              bcs[:])
            yield

    # ============ dt factory: one tile -> dts ring slot ==============
    def emit_factory(bname, mi, ci, dts_ring):
        xcb, _, _ = tiles[bname]
        c0 = ci * CT
        dtin = rg.tile([8, CT], bf16, name="dtin", tag="dtin", bufs=2)
        nc.sync.dma_start(dtin[:],
                          aps[f"scr_bc_{bname}"][0:8, c0:c0 + CT])
        pd = psA("pd")
        for cc in range(2):
            mm(pd[:, cc * CT:(cc + 1) * CT], BR(f"dtw{mi}{cc}"),
               dtin[:], True, True, kp=8)
        ez = rg.tile([128, 2 * CT], bf16, name="ez", tag="ez", bufs=1)
        if flags["dtb_const"] is not None:
            nc.scalar.activation(ez[:], pd[:], AF.Exp, bias=VP("dtbc"))
        else:
            for cc in range(2):
                nc.scalar.activation(
                    ez[:, cc * CT:(cc + 1) * CT],
                    pd[:, cc * CT:(cc + 1) * CT],
                    AF.Exp, bias=VP(f"dtb{mi}{cc}"))
        dts = rg.tile([128, 2 * CT], bf16, name="dts", tag="dts", bufs=3)
        nc.scalar.activation(dts[:], ez[:], AF.Ln, bias=1.0)
        dtx = rg.tile([128, 2 * CT], bf16, name="dtx", tag="dtx", bufs=2)
        for cc in range(2):
            nc.vector.tensor_mul(dtx[:, cc * CT:(cc + 1) * CT],
                                 dts[:, cc * CT:(cc + 1) * CT],
                                 xcb[cc][:, c0:c0 + CT])
        for cc in range(2):
            nc.sync.dma_start(
                aps[f"scr_dtx_{bname}"][cc, ci]
                .rearrange("p (g c) -> g p c", g=16),
                dtx[:, cc * CT:(cc + 1) * CT])
        dtv = dts[:].rearrange("p (x t) -> p x t", t=W)
        nc.vector.tensor_scalar(dtv[:, :, 0:1], dtv[:, :, 0:1],
                                0.0, 1.0e4, OP.mult, OP.add)
        dts_ring[ci] = dts

    # ============ scan unit (one tile) ===============================
    def emit_scan(bname, mi, ci, dts_ring, is_t, s_tile, s_c0):
        xcb, zsb, xT = tiles[bname]
        c0 = ci * CT
        dts = dts_ring.pop(ci)
        Brep = rg.tile([128, CT], bf16, name="Brep", tag="Brep", bufs=2)
        nc.sync.dma_start(
            Brep[:],
            aps[f"scr_bc_{bname}"][8:24, c0:c0 + CT]
            .unsqueeze(0).broadcast_to((8, 16, CT)))
        Crep = rg.tile([128, CT], bf16, name="Crep", tag="Crep", bufs=2)
        nc.sync.dma_start(
            Crep[:],
            aps[f"scr_bc_{bname}"][24:40, c0:c0 + CT]
            .unsqueeze(0).broadcast_to((8, 16, CT)))
        ues = []
        for cc in range(2):
            halves = []
            for hf in range(2):
                ueh = rg.tile([128, 8 * CT], bf16, name="ueh",
                              tag=f"ueh{hf}", bufs=2 - hf)
                nc.sync.dma_start(
                    ueh[:],
                    aps[f"scr_dtx_{bname}"]
                    [cc, ci, :, hf * 8 * CT:(hf + 1) * 8 * CT]
                    .unsqueeze(1).broadcast_to((8, 16, 8 * CT)))
                halves.append(ueh)
            ues.append(halves)
        Brep_b = Brep[:].unsqueeze(1).broadcast_to((128, 2, CT))
        Crep_b = Crep[:].unsqueeze(1).broadcast_to((128, 2, CT))
        gzs = []
        for cc in range(2):
            pY = psY(f"pY{cc}")
            stage = []
            for gp in range(11):
                if gp < 8:
                    pP = psA("pP")
                    for i in range(2):
                        g = gp * 2 + i
                        mm(pP[:, i * CT:(i + 1) * CT],
                           dAW(mi, cc, g),
                           dts[:, cc * CT:(cc + 1) * CT],
                           True, True)
                    dA = rg.tile([128, 2 * CT], bf16, name="dA",
                                 tag="dA", bufs=3)
                    nc.scalar.activation(dA[:], pP[:], AF.Exp)
                    u2 = rg.tile([128, 2 * CT], bf16, name="u2",
                                 tag="u2", bufs=3)
                    ueh = ues[cc][gp // 4]
                    sl = (gp % 4) * 2 * CT
                    eng = nc.gpsimd if gp % 4 == 3 else nc.vector
                    eng.tensor_mul(
                        u2[:].rearrange("p (i c) -> p i c", i=2),
                        ueh[:, sl:sl + 2 * CT]
                        .rearrange("p (i c) -> p i c", i=2),
                        Brep_b)
                    stage.append((gp, dA, u2))
                if gp >= 3:
                    gq, dAq, u2q = stage.pop(0)
                    h2 = rg.tile([128, 2 * CT], bf16, name="h2",
                                 tag="h2", bufs=2)
                    for i in range(2):
                        nc.vector.tensor_tensor_scan(
                            h2[:, i * CT:(i + 1) * CT],
                            dAq[:, i * CT:(i + 1) * CT],
                            u2q[:, i * CT:(i + 1) * CT],
                            0.0, OP.mult, OP.add)
                    yh2 = rg.tile([128, 2 * CT], bf16, name="yh2",
                                  tag="yh2", bufs=2)
                    eng = nc.gpsimd if gq % 2 == 1 else nc.vector
                    eng.tensor_mul(
                        yh2[:].rearrange("p (i c) -> p i c", i=2),
                        h2[:].rearrange("p (i c) -> p i c", i=2),
                        Crep_b)
                    for i in range(2):
                        g = gq * 2 + i
                        mm(pY[:], BR(f"sum{g}"),
                           yh2[:, i * CT:(i + 1) * CT],
                           g == 0, g == 15)
            yg = rg.tile([128, CT], bf16, name=f"yg{cc}", tag=f"yg{cc}",
                         bufs=1)
            if flags["D_ones"]:
                nc.vector.tensor_add(yg[:], pY[:],
                                     xcb[cc][:, c0:c0 + CT])
            else:
                nc.vector.scalar_tensor_tensor(
                    yg[:], xcb[cc][:, c0:c0 + CT],
                    VP(f"Dp{mi}{cc}"), pY[:], OP.mult, OP.add)
            gz = rg.tile([128, CT], bf16, name=f"gz{cc}", tag=f"gz{cc}",
                         bufs=1)
            nc.gpsimd.tensor_mul(gz[:], yg[:], zsb[cc][:, c0:c0 + CT])
            gzs.append(gz)
        po = psC("po")
        for cc in range(2):
            mm(po[:], BR(f"wout{mi}{cc}"), gzs[cc][:], cc == 0, cc == 1)
        res = s_tile[:, s_c0:s_c0 + CT]
        if is_t:
            nc.vector.tensor_add(res, po[:], xT[:, c0:c0 + CT])
            s2 = rg.tile([128, CT], bf16, name="s2", tag="s2", bufs=1)
            nc.gpsimd.tensor_mul(s2[:], res, res)
            ln_stats(res, s2[:], (bname, "a"), c0)
            ln_finish((bname, "a"), c0)
        else:
            # n/l: write pre-mix value (ym) into the s tile; mix rewrites
            nc.scalar.activation(res, po[:], AF.Copy)

    # ========= factory + scan generator =====================
    # units: fac(0), fac(1), [scan(0), fac(2)], [scan(1), fac(3)], ...
    def g_fs(bname):
        _, mi, _, _, _ = HB[bname]
        is_t = bname[0] == "t"
        s_tile = s_full.get(bname)
        dts_ring = {}
        emit_factory(bname, mi, 0, dts_ring)
        yield
        emit_factory(bname, mi, 1, dts_ring)
        yield
        for ci in range(NT):
            if s_tile is not None:
                st, sc0 = s_tile, ci * CT
            else:
                st = rg.tile([128, CT], bf16, name="sT", tag="sT", bufs=8)
                tiles_s[(bname, ci)] = st
                sc0 = 0
            emit_scan(bname, mi, ci, dts_ring, is_t, st, sc0)
            if ci + 2 < NT:
                emit_factory(bname, mi, ci + 2, dts_ring)
            yield

    tiles_s = {}

    # ================= mix generator (8 units) =======================
    def g_mix(pair):
        nb, lb = pair
        q_n = HB[nb][3]
        q_l = HB[lb][3]
        for c0 in range(0, CBLK, CT):
            cat = {"n": s_full[nb][:, c0:c0 + CT],
                   "l": s_full[lb][:, c0:c0 + CT]}
            # both mix matmuls first: they read cat slices that the res
            # writes below overwrite in place (s tile doubles as ym)
            mss = []
            for mc in range(2):
                pmx = psC("pmx")
                for kc, kk in enumerate(("n", "l")):
                    mm(pmx[:], BR(f"mix{kc}{mc}"), cat[kk], kc == 0,
                       kc == 1)
                ms = rg.tile([128, CT], bf16, name="ms", tag=f"ms{mc}",
                             bufs=1)
                if flags["mixb_zero"]:
                    nc.scalar.activation(ms[:], pmx[:], AF.Silu)
                else:
                    nc.scalar.activation(ms[:], pmx[:], AF.Silu,
                                         bias=VP(f"mixb{mc}"))
                mss.append(ms)
            # t2 adds also read cat before any res write
            t2s = []
            for mc, key in enumerate(("n", "l")):
                t2 = rg.tile([128, CT], bf16, name="t2", tag=f"t2{mc}",
                             bufs=1)
                nc.vector.tensor_add(t2[:], cat[key], mss[mc][:])
                t2s.append(t2)
            for mc, (key, bn, xk, qo) in enumerate(
                    (("n", nb, "x_n", q_n), ("l", lb, "x_l", q_l))):
                q0 = qo + c0 // W
                raw = rg.tile([64, 8 * 128], f32, name="rawm", tag="raw",
                              bufs=2)
                nc.sync.dma_start(raw[:], aps[xk][:, q0:q0 + 8, :])
                ptx = psC("ptx")
                for i in range(8):
                    nc.tensor.transpose(ptx[:, i * 64:(i + 1) * 64],
                                        raw[:, i * 128:(i + 1) * 128],
                                        I64[:64, :64])
                res = s_full[bn][:, c0:c0 + CT]
                nc.vector.tensor_add(res, t2s[mc][:], ptx[:])
                s2m = rg.tile([128, CT], bf16, name="s2m", tag="s2m",
                              bufs=2)
                nc.gpsimd.tensor_mul(s2m[:], res, res)
                ln_stats(res, s2m[:], (bn, "a"), c0)
            yield

    # ================= phase C generator (16 units) ==================
    def g_C(bname, fin_a):
        _, mi, _, _, j0 = HB[bname]
        is_t = bname[0] == "t"

        def CF(ci):
            c0 = ci * CT
            if is_t:
                src = tiles_s.pop((bname, ci))[:]
            else:
                src = s_full[bname][:, c0:c0 + CT]
            n1 = rg.tile([128, CT], bf16, name="n1", tag="n1", bufs=2)
            ga, gb = (None, None) if flags["an_id"] else \
                (f"ang{mi}", f"anb{mi}")
            ln_apply(src, (bname, "a"), c0, ga, gb, n1[:])
            hh = rg.tile([128, 4 * CT], bf16, name="hh", tag="hh", bufs=1)
            for hp in range(2):
                pf = psA("pf")
                for ci2 in range(2):
                    c4 = hp * 2 + ci2
                    mm(pf[:, ci2 * CT:(ci2 + 1) * CT],
                       BR(f"ff1{mi}")[:, c4 * 128:(c4 + 1) * 128],
                       n1[:], True, True)
                if flags["f1b_zero"]:
                    nc.scalar.activation(
                        hh[:, hp * 2 * CT:(hp + 1) * 2 * CT], pf[:],
                        AF.Lrelu, alpha=0.01)
                else:
                    for ci2 in range(2):
                        c4 = hp * 2 + ci2
                        nc.scalar.activation(
                            hh[:, c4 * CT:(c4 + 1) * CT],
                            pf[:, ci2 * CT:(ci2 + 1) * CT], AF.Lrelu,
                            bias=VP(f"f1b{mi}{c4}"), alpha=0.01)
            pf2 = psC("pf2")
            for c4 in range(4):
                mm(pf2[:], BR(f"ff2{mi}{c4}"),
                   hh[:, c4 * CT:(c4 + 1) * CT], c4 == 0, c4 == 3)
            sf = rg.tile([128, CT], bf16, name="sf", tag="sf", bufs=2)
            if flags["f2b_zero"]:
                nc.vector.tensor_add(sf[:], pf2[:], n1[:])
            else:
                nc.vector.scalar_tensor_tensor(sf[:], pf2[:],
                                               VP(f"f2b{mi}"),
                                               n1[:], OP.add, OP.add)
            s2f = rg.tile([128, CT], bf16, name="s2f", tag="s2f", bufs=1)
            nc.gpsimd.tensor_mul(s2f[:], sf[:], sf[:])
            ln_stats(sf[:], s2f[:], (bname, "f"), c0)
            return sf

        def CT_out(ci, sf):
            c0 = ci * CT
            ga, gb = (None, None) if flags["fln_id"] else \
                (f"flg{mi}", f"flb{mi}")
            n2 = rg.tile([128, CT], f32, name="n2", tag="n2", bufs=1)
            ln_apply(sf[:], (bname, "f"), c0, ga, gb, n2[:])
            pto = psC("pto")
            for c in range(4):
                nc.tensor.transpose(pto[:, c * 128:(c + 1) * 128],
                                    n2[:, c * 128:(c + 1) * 128],
                                    I128)
            ot = rg.tile([128, CT], f32, name="ot", tag="ot", bufs=1)
            nc.vector.tensor_copy(ot[:], pto[:])
            q0 = c0 // W
            for qh in range(2):
                nc.sync.dma_start(
                    aps["out"][:, j0 + q0:j0 + q0 + 8, :]
                    .rearrange("t (c q) d -> q t c d", c=4)[qh],
                    ot[qh * 64:(qh + 1) * 64, :]
                    .rearrange("t (c d) -> t c d", c=4))

        # software-pipelined: finishes run one tile ahead of the applies
        # that consume their DRAM-bounced scale rows, so the rnm2 load DMA
        # never holds the DMA queue waiting on a just-issued write.
        if fin_a:
            ln_finish((bname, "a"), 0)
        sf_prev = None
        for ci in range(NT):
            if is_t:
                while (bname, ci) not in tiles_s:
                    yield   # scan hasn't produced this tile yet; spin
            if fin_a and ci + 1 < NT:
                ln_finish((bname, "a"), (ci + 1) * CT)
            sf = CF(ci)
            if ci >= 1:
                ln_finish((bname, "f"), (ci - 1) * CT)
            yield
            if sf_prev is not None:
                CT_out(ci - 1, sf_prev)
                yield
            sf_prev = sf
        ln_finish((bname, "f"), (NT - 1) * CT)
        yield
        CT_out(NT - 1, sf_prev)
        yield

    # ===================== master schedule ===========================
    with tc.tile_pool(name="a_sq", bufs=1) as a_sqp:
        cq = []   # persistent queue of pending C generators

        def A1(bn):
            return g_a1(bn, a_sqp)

        # order: n0, l0, t0, n1, l1, t1, t2, t3 — a family's C phase fully
        # drains inside t-block windows before its s/sT tag is reused.
        _drain(A1("n0"))
        _drain(g_fs("n0"))
        _drain(A1("l0"))
        _drain(g_fs("l0"))
        _weave([(A1("t0"), 1), (g_mix(("n0", "l0")), 1)])
        C_n0 = GStream(g_C("n0", True))
        C_l0 = GStream(g_C("l0", True))
        C_t0 = GStream(_delay(5, g_C("t0", False)))
        cq.extend([C_n0, C_l0, C_t0])
        _weave_until(g_fs("t0"), cq, w_master=1, w_c=3)
        C_n0.finish()          # s_n tag is rewritten by scan(n1)
        _drain(A1("n1"))       # silu window: no C (act-table isolation)
        _weave_until(g_fs("n1"), cq, w_master=1, w_c=3)
        C_l0.finish()          # s_l tag is rewritten by scan(l1)
        _drain(A1("l1"))
        _weave_until(g_fs("l1"), cq, w_master=1, w_c=3)
        _weave([(A1("t1"), 1), (g_mix(("n1", "l1")), 1)])
        C_prev = C_t0
        cq.extend([GStream(g_C("n1", True)), GStream(g_C("l1", True))])
        for bn, nxt in (("t1", "t2"), ("t2", "t3"), ("t3", None)):
            C_prev.finish()    # sT ring slots reused by scan(bn)
            C_cur = GStream(_delay(5, g_C(bn, False)))
            cq.append(C_cur)
            _weave_until(g_fs(bn), cq, w_master=1, w_c=4)
            if nxt is not None:
                _drain(A1(nxt))
            C_prev = C_cur
        for g in cq:
            g.finish()


def _build_program(wp, vp, bpk, flags):
    nc = bacc.Bacc("TRN2", target_bir_lowering=False, debug=False,
                   num_devices=N_CORES)
    aps = {}
    aps["x_n"] = nc.dram_tensor("x_n", [W, N, D], f32,
                                kind="ExternalInput").ap()
    aps["x_t"] = nc.dram_tensor("x_t", [W, E, D], f32,
                                kind="ExternalInput").ap()
    aps["x_l"] = nc.dram_tensor("x_l", [W, N, D], f32,
                                kind="ExternalInput").ap()
    aps["wpack"] = nc.dram_tensor("wpack", [128, wp.n], f32,
                                  kind="ExternalInput").ap()
    aps["vpack"] = nc.dram_tensor("vpack", [128, vp.n], f32,
                                  kind="ExternalInput").ap()
    aps["bpack"] = nc.dram_tensor("bpack", [128, bpk.n], bf16,
                                  kind="ExternalInput").ap()
    aps["out"] = nc.dram_tensor("out", [W, 2 * N + E, D], f32,
                                kind="ExternalOutput").ap()
    for bname, _, _, _, _ in HBLOCKS:
        aps[f"scr_bc_{bname}"] = nc.dram_tensor(
            f"scr_bc_{bname}", [40, CBLK], bf16).ap()
        aps[f"scr_dtx_{bname}"] = nc.dram_tensor(
            f"scr_dtx_{bname}", [2, NT, 8, 16 * CT], bf16).ap()
        for ph in ("a", "f"):
            aps[f"scr_rn_{bname}_{ph}"] = nc.dram_tensor(
                f"scr_rn_{bname}_{ph}", [2, 128, NW], bf16).ap()

    with tile.TileContext(nc) as tc:
        with ExitStack() as ctx:
            _emit(ctx, tc, nc, aps, wp, vp, bpk, flags)
    nc.compile()
    return nc


_CACHE = {}


def kernel(**inputs):
    wp, vp, bpk, flags = _host_pack(inputs)
    if "prog" not in _CACHE:
        _CACHE["prog"] = _build_program(wp, vp, bpk, flags)
    nc = _CACHE["prog"]
    wpack, vpack = wp.build(), vp.build()
    bpack = bpk.build().astype(ml_dtypes.bfloat16)
    in_maps = []
    for b in range(B):
        in_maps.append({
            "x_n": np.ascontiguousarray(inputs["x_node"][b]),
            "x_t": np.ascontiguousarray(inputs["x_trace"][b]),
            "x_l": np.ascontiguousarray(inputs["x_log"][b]),
            "wpack": wpack,
            "vpack": vpack,
            "bpack": bpack,
        })
    res = run_bass_kernel_spmd(nc, in_maps, list(range(N_CORES)))
    out = np.stack([res.results[b]["out"] for b in range(B)], axis=0)
    return out.astype(np.float32)


# revision 24
# speedup vs baseline: 1.1345x; 1.0466x over previous
"""Trainium2 Bass kernel for nn_Encoder (tri-modal Mamba encoder), v3.

kernel(**inputs) takes FULL unsharded numpy inputs and returns the FULL
output (B, W, 2N+E, D). Batch B=8 is sharded across 8 NeuronCores (pure
data parallel, no collectives); params are replicated.

v3 vs v2 (same math, new orchestration):
- Half-block (64-seq) processing units, software-pipelined end to end:
  the AddNorm+FFN+output phase (C) of each half-block is emitted as a
  persistent generator that drains into whatever later window has engine
  slack, so the serial phase-C tail is gone.
- LayerNorm stats finish per column-tile with rsqrt = exp(-0.5*ln(v+eps))
  so interleaved phases stay inside the natural_log_exp activation-table
  set (Silu windows are kept separate: A1 and mix).
- s / LN stats / LN scale factors live in SBUF; ym merged into s tiles.
- One shared 8-bank PSUM tag discipline (psA 2x[128,1024]f32,
  psY 2x[128,512], psC 2x[128,512]) across all phases.
- Output path: 4 batched f32 transposes per 512-col tile, one copy, one
  DMA (q t d scatter) straight from the LN-f apply.
"""

import functools

import ml_dtypes
import numpy as np
from contextlib import ExitStack

import concourse.bass as bass
import concourse.tile as tile
import concourse.bacc as bacc_mod
import concourse.hw_specs as hw_specs_mod
from concourse import bacc, mybir
from concourse.bass_utils import run_bass_kernel_spmd

# Prefer the exp+ln combined activation-table set so the Exp/Ln mix
# (softplus factory, scan decay, LN rsqrt) resolves to ONE table and the
# act-table load pass stops thrashing between exp_and_others/natural_log.
_GAT_ORIG = hw_specs_mod.get_activation_tables.__wrapped__


@functools.cache
def _gat_reordered(arch):
    t = dict(_GAT_ORIG(arch))
    pref = [k for k in ("natural_log_exp_and_others",) if k in t]
    return {k: t[k] for k in pref + [k for k in t if k not in pref]}


hw_specs_mod.get_activation_tables = _gat_reordered
bacc_mod.get_activation_tables = _gat_reordered

D, DI, SS, KK, RR = 128, 256, 16, 4, 8
B, W, N, E = 8, 64, 128, 256
Q = 64                       # seqs per half-block
CBLK = Q * W                 # 4096 cols per half-block
CT = 512                     # column tile (8 seqs)
NT = CBLK // CT              # 8 tiles per half-block
NW = CBLK // 128             # 32 stat cols per partition
f32 = mybir.dt.float32
f32r = mybir.dt.float32r
bf16 = mybir.dt.bfloat16
AF = mybir.ActivationFunctionType
OP = mybir.AluOpType

# (name, modality, input key, q offset, output entity offset)
HBLOCKS = [("n0", 0, "x_n", 0, 0), ("n1", 0, "x_n", 64, 64),
           ("l0", 2, "x_l", 0, 384), ("l1", 2, "x_l", 64, 448),
           ("t0", 1, "x_t", 0, 128), ("t1", 1, "x_t", 64, 192),
           ("t2", 1, "x_t", 128, 256), ("t3", 1, "x_t", 192, 320)]
HB = {b[0]: b for b in HBLOCKS}
N_CORES = 8
LN_EPS = 1e-5


class Pack:
    def __init__(self):
        self.cols = []
        self.off = {}
        self.n = 0

    def add(self, name, arr, dtype=np.float32):
        arr = np.asarray(arr, dtype)
        assert arr.ndim == 2 and arr.shape[0] <= 128
        a = np.zeros((128, arr.shape[1]), dtype)
        a[: arr.shape[0]] = arr
        self.off[name] = (self.n, arr.shape[1])
        self.cols.append(a)
        self.n += arr.shape[1]

    def build(self):
        return np.concatenate(self.cols, axis=1)


def _host_pack(inp):
    """Returns (wp f32-staged-to-f32r, vp f32, bp bf16, flags)."""
    flags = {}
    dtb = np.asarray(inp["mp_dt_b"], np.float64)
    flags["dtb_const"] = float(dtb.flat[0]) if np.ptp(dtb) < 1e-12 else None
    flags["D_ones"] = bool(np.allclose(np.asarray(inp["mp_D"]), 1.0))
    flags["convb_zero"] = bool(np.all(np.asarray(inp["mp_conv_b"]) == 0.0))
    flags["f1b_zero"] = bool(np.all(np.asarray(inp["ff1_b"]) == 0.0))
    flags["f2b_zero"] = bool(np.all(np.asarray(inp["ff2_b"]) == 0.0))
    flags["an_id"] = bool(np.all(np.asarray(inp["an_g"]) == 1.0)
                          and np.all(np.asarray(inp["an_b"]) == 0.0))
    flags["fln_id"] = bool(np.all(np.asarray(inp["fln_g"]) == 1.0)
                           and np.all(np.asarray(inp["fln_b"]) == 0.0))
    flags["mixb_zero"] = bool(np.all(np.asarray(inp["mix_b"]) == 0.0))
    A = -np.exp(np.asarray(inp["mp_Alog"], np.float64))      # (3, DI, S)
    flags["A_shared"] = bool(
        np.ptp(A, axis=(0, 1)).max() < 1e-9 * np.abs(A).max())

    bp = Pack()   # bf16 weights
    for g in range(16):
        sm = np.zeros((128, 128), np.float32)
        for k in range(128):
            sm[k, g * 8 + k // 16] = 1.0
        bp.add(f"sum{g}", sm)

    def delta_A(Am, cc, g):
        dl = np.zeros((128, 128), np.float32)
        for j in range(128):
            dl[g * 8 + j // 16, j] = Am[cc * 128 + g * 8 + j // 16, j % 16]
        return dl

    if flags["A_shared"]:
        for g in range(16):
            bp.add(f"dA{g}", delta_A(A[0], 0, g))
    else:
        for m in range(3):
            for cc in range(2):
                for g in range(16):
                    bp.add(f"dA{m}{cc}{g}", delta_A(A[m], cc, g))

    wp = Pack()   # fp32 staged -> f32r on device
    vp = Pack()   # fp32 per-partition vectors
    for m in range(3):
        bp.add(f"win{m}", inp["mp_in"][m])                   # (D, 512)
        wxp = inp["mp_xproj"][m]                             # (DI, 40)
        for cc in range(2):
            bp.add(f"bc{m}{cc}", wxp[cc * 128:(cc + 1) * 128])
        dtw = inp["mp_dt_w"][m]                              # (R, DI)
        for cc in range(2):
            bp.add(f"dtw{m}{cc}", dtw[:, cc * 128:(cc + 1) * 128])
        wout = inp["mp_out"][m]                              # (DI, D)
        for cc in range(2):
            bp.add(f"wout{m}{cc}", wout[cc * 128:(cc + 1) * 128])
        bp.add(f"ff1{m}", inp["ff1_w"][m])                   # (D, 512)
        ff2 = inp["ff2_w"][m]                                # (4D, D)
        for c4 in range(4):
            bp.add(f"ff2{m}{c4}", ff2[c4 * 128:(c4 + 1) * 128])
    mixw = inp["mix_w"]
    for kc in range(2):
        for mc in range(2):
            bp.add(f"mix{kc}{mc}", mixw[kc * 128:(kc + 1) * 128,
                                        mc * 128:(mc + 1) * 128])
    wp.add("onesD", np.full((128, 1), 1.0 / D, np.float32))
    bp.add("onesDb", np.full((128, 1), 1.0 / D, np.float32))
    wp.add("ones1", np.ones((1, 128), np.float32))

    vp.add("eps", np.full((128, 1), LN_EPS, np.float32))
    if flags["dtb_const"] is not None:
        vp.add("dtbc", np.full((128, 1), flags["dtb_const"], np.float32))
    vp.add("I64", np.eye(64, dtype=np.float32))
    vp.add("I128", np.eye(128, dtype=np.float32))
    for m in range(3):
        cw = inp["mp_conv_w"][m]
        for cc in range(2):
            sl = slice(cc * 128, (cc + 1) * 128)
            vp.add(f"cw{m}{cc}", cw[sl])                     # 4 cols
            if not flags["convb_zero"]:
                vp.add(f"cb{m}{cc}", inp["mp_conv_b"][m][sl, None])
            if flags["dtb_const"] is None:
                vp.add(f"dtb{m}{cc}", inp["mp_dt_b"][m][sl, None])
            if not flags["D_ones"]:
                vp.add(f"Dp{m}{cc}", inp["mp_D"][m][sl, None])
        if not flags["f1b_zero"]:
            for c4 in range(4):
                vp.add(f"f1b{m}{c4}",
                       inp["ff1_b"][m][c4 * 128:(c4 + 1) * 128, None])
        if not flags["f2b_zero"]:
            vp.add(f"f2b{m}", inp["ff2_b"][m][:, None])
        if not flags["an_id"]:
            vp.add(f"ang{m}", inp["an_g"][m][:, None])
            vp.add(f"anb{m}", inp["an_b"][m][:, None])
        if not flags["fln_id"]:
            vp.add(f"flg{m}", inp["fln_g"][m][:, None])
            vp.add(f"flb{m}", inp["fln_b"][m][:, None])
    if not flags["mixb_zero"]:
        for mc in range(2):
            vp.add(f"mixb{mc}", inp["mix_b"][mc * 128:(mc + 1) * 128, None])
    return wp, vp, bp, flags


def _drain(g):
    for _ in g:
        pass


def _weave(streams):
    """streams: (gen, weight) pairs; round-robin to exhaustion."""
    live = [[iter(g), w] for g, w in streams]
    while live:
        for ent in list(live):
            g, w = ent
            for _ in range(w):
                try:
                    next(g)
                except StopIteration:
                    live.remove(ent)
                    break


class GStream:
    def __init__(self, g):
        self.g = iter(g)
        self.done = False

    def step(self):
        if self.done:
            return False
        try:
            next(self.g)
            return True
        except StopIteration:
            self.done = True
            return False

    def finish(self):
        while self.step():
            pass


def _weave_until(master, cs, w_master=1, w_c=1):
    """Interleave master with the persistent queue `cs` (GStream list).
    Returns when master is exhausted; cs keeps its remaining state."""
    m = iter(master)
    while True:
        for _ in range(w_master):
            try:
                next(m)
            except StopIteration:
                return
        budget = w_c
        while budget > 0 and cs:
            if cs[0].step():
                budget -= 1
            else:
                cs.pop(0)


def _delay(n, g):
    for _ in range(n):
        yield
    yield from g


def _emit(ctx, tc, nc, aps, wp, vp, bpk, flags):
    wpool = ctx.enter_context(tc.tile_pool(name="weights", bufs=1))
    wr = wpool.tile([128, wp.n], f32r, name="wr", tag="wr")
    vec = wpool.tile([128, vp.n], f32, name="vec", tag="vec")
    nc.sync.dma_start(vec[:], aps["vpack"][:])
    bw = wpool.tile([128, bpk.n], bf16, name="bw", tag="bw")
    nc.sync.dma_start(bw[:], aps["bpack"][:])
    with tc.tile_pool(name="wstage", bufs=1) as stpool:
        wstage = stpool.tile([128, wp.n], f32, name="wstage")
        nc.sync.dma_start(wstage[:], aps["wpack"][:])
        for o in range(0, wp.n, 8192):
            e = min(wp.n, o + 8192)
            nc.vector.tensor_copy(wr[:, o:e], wstage[:, o:e])

    def WR(name):
        o, c = wp.off[name]
        return wr[:, o:o + c]

    def VP(name):
        o, c = vp.off[name]
        return vec[:, o:o + c]

    def BR(name):
        o, c = bpk.off[name]
        return bw[:, o:o + c]

    def mm(psum_ap, lhsT_ap, rhs_ap, start, stop, kp=128):
        nc.tensor.matmul(psum_ap, lhsT_ap[:kp, :], rhs_ap[:kp, :],
                         start=start, stop=stop)

    def dAW(mi, cc, g):
        return BR(f"dA{g}" if flags["A_shared"] else f"dA{mi}{cc}{g}")

    I64 = VP("I64")
    I128 = VP("I128")

    # ---- long-lived SBUF state --------------------------------------
    blk = ctx.enter_context(tc.tile_pool(name="blk", bufs=1))
    # s tiles: n/l halves need full (128, CBLK); mix writes them in place
    # over the ym values (same storage). t halves use a small ring.
    s_full = {nm: blk.tile([128, CBLK], bf16, name=f"s_{nm}",
                           tag=f"s_{nm[0]}")
              for nm in ("n0", "n1", "l0", "l1")}  # tag per family: 2 tags
    statp = ctx.enter_context(tc.tile_pool(name="stat", bufs=1))
    stat, rnm = {}, {}
    for bname, _, _, _, _ in HBLOCKS:
        for ph in ("a", "f"):
            stat[(bname, ph)] = statp.tile(
                [128, 2 * NW], f32, name=f"st_{bname}{ph}",
                tag=f"st_{bname}{ph}")
            rnm[(bname, ph)] = statp.tile(
                [128, 2 * NW], bf16, name=f"rn_{bname}{ph}",
                tag=f"rn_{bname}{ph}")

    # ---- shared PSUM tags (8 banks total) ---------------------------
    ps = ctx.enter_context(tc.tile_pool(name="ps", bufs=1, space="PSUM"))

    def psA(name):
        return ps.tile([128, 2 * CT], f32, name=name, tag="psA", bufs=2)

    def psS(name):
        return ps.tile([1, 2 * CT], f32, name=name, tag="psA", bufs=2)

    def psY(name):
        return ps.tile([128, CT], f32, name=name, tag="psY", bufs=2)

    def psC(name):
        return ps.tile([128, CT], f32, name=name, tag="psC", bufs=2)

    rg = ctx.enter_context(tc.tile_pool(name="rg", bufs=1))

    # ---- LN helpers -------------------------------------------------
    def ln_stats(src_ap, sq_ap, statT, c0):
        ob = BR("onesDb")[:, 0:1]
        of = WR("onesD")[:, 0:1]
        pmq = psS("pmq")
        mm(pmq[:, 0:CT], ob if src_ap.dtype == bf16 else of, src_ap,
           True, True)
        mm(pmq[:, CT:2 * CT], ob if sq_ap.dtype == bf16 else of, sq_ap,
           True, True)
        sst = rg.tile([1, 2 * CT], f32, name="sst", tag="sst", bufs=1)
        nc.scalar.activation(sst[:], pmq[:], AF.Copy)
        p0 = (c0 // CT) * 16
        nc.sync.dma_start(
            stat[statT][p0:p0 + 16, :].rearrange("p (h w) -> h p w", h=2),
            sst[:].rearrange("x (h p w) -> x h p w", h=2, p=16))

    def ln_finish(statT, c0):
        """Finish LN scale factors for one column tile (16 partitions)."""
        p0 = (c0 // CT) * 16
        sT = stat[statT]
        rT = rnm[statT]
        m_t = sT[p0:p0 + 16, 0:NW]
        q_t = sT[p0:p0 + 16, NW:2 * NW]
        var = rg.tile([16, NW], f32, name="var", tag="lnvar", bufs=2)
        nc.vector.tensor_mul(var[:], m_t, m_t)
        nc.vector.tensor_sub(var[:], q_t, var[:])
        lnv = rg.tile([16, NW], f32, name="lnv", tag="lnlnv", bufs=2)
        nc.scalar.activation(lnv[:], var[:], AF.Ln, bias=VP("eps")[0:16])
        r_t = rg.tile([16, NW], f32, name="lnr", tag="lnr", bufs=2)
        nc.scalar.activation(r_t[:], lnv[:], AF.Exp, scale=-0.5)
        nmr = rg.tile([16, NW], f32, name="nmr", tag="lnnmr", bufs=2)
        nc.vector.tensor_mul(nmr[:], m_t, r_t[:])
        nc.vector.tensor_scalar(nmr[:], nmr[:], -1.0, None, OP.mult)
        nc.vector.tensor_copy(rT[p0:p0 + 16, 0:NW], r_t[:])
        nc.vector.tensor_copy(rT[p0:p0 + 16, NW:2 * NW], nmr[:])
        bn, ph = statT
        nc.sync.dma_start(
            aps[f"scr_rn_{bn}_{ph}"][:, p0:p0 + 16, :],
            rT[p0:p0 + 16, :].rearrange("p (h w) -> h p w", h=2))

    def ln_apply(src_ap, rnmT, c0, gk, bk, out_ap):
        p0 = (c0 // CT) * 16
        rnm2 = rg.tile([128, 2 * CT], bf16, name="rnm2", tag="rnm2",
                       bufs=2)
        bn, ph = rnmT
        nc.sync.dma_start(
            rnm2[:].rearrange("j (h ab) -> j h ab", h=2),
            aps[f"scr_rn_{bn}_{ph}"][:, p0:p0 + 16, :]
            .rearrange("h p w -> h (p w)")
            .unsqueeze(0).broadcast_to((128, 2, CT)))
        t1 = rg.tile([128, CT], bf16, name="t1", tag="t1", bufs=2)
        nc.vector.tensor_mul(t1[:], src_ap, rnm2[:, 0:CT])
        if gk is None:
            nc.vector.tensor_add(out_ap, t1[:], rnm2[:, CT:2 * CT])
        else:
            nc.vector.tensor_add(t1[:], t1[:], rnm2[:, CT:2 * CT])
            nc.vector.tensor_scalar(out_ap, t1[:], VP(gk), VP(bk),
                                    OP.mult, OP.add)

    tiles = {}

    # ================= phase A1 (8 units) ============================
    def g_a1(bname, sqp):
        _, mi, xkey, q_off, _ = HB[bname]
        xcb = [sqp.tile([128, CBLK], bf16, name=f"xcb{cc}", tag=f"xcb{cc}")
               for cc in range(2)]
        zsb = [sqp.tile([128, CBLK], bf16, name=f"zsb{cc}", tag=f"zsb{cc}")
               for cc in range(2)]
        xT = sqp.tile([128, CBLK], bf16, name="xT", tag="xT")
        tiles[bname] = (xcb, zsb, xT)
        for c0 in range(0, CBLK, CT):
            q0 = c0 // W
            raw = rg.tile([64, 8 * 128], f32, name="raw", tag="raw", bufs=2)
            nc.sync.dma_start(raw[:],
                              aps[xkey][:, q_off + q0:q_off + q0 + 8, :])
            pt = psC("pt")
            for i in range(8):
                nc.tensor.transpose(pt[:, i * 64:(i + 1) * 64],
                                    raw[:, i * 128:(i + 1) * 128],
                                    I64[:64, :64])
            nc.scalar.activation(xT[:, c0:c0 + CT], pt[:], AF.Copy)
            xt_t = xT[:, c0:c0 + CT]
            pxc2 = psA("pxc2")
            for cc in range(2):
                mm(pxc2[:, cc * CT:(cc + 1) * CT],
                   BR(f"win{mi}")[:, cc * 128:(cc + 1) * 128], xt_t,
                   True, True)
            pz2 = psA("pz2")
            for cc in range(2):
                mm(pz2[:, cc * CT:(cc + 1) * CT],
                   BR(f"win{mi}")[:, (2 + cc) * 128:(3 + cc) * 128],
                   xt_t, True, True)
            for cc in range(2):
                nc.scalar.activation(zsb[cc][:, c0:c0 + CT],
                                     pz2[:, cc * CT:(cc + 1) * CT],
                                     AF.Silu)
            zc = rg.tile([128, 2 * CT], bf16, name="zc", tag="zc", bufs=1)
            nc.scalar.activation(zc[:], pxc2[:], AF.Copy)
            acc2 = rg.tile([128, 2 * CT], f32, name="acc2", tag="acc2",
                           bufs=1)
            for cc in range(2):
                pzv = zc[:, cc * CT:(cc + 1) * CT]
                accv = acc2[:, cc * CT:(cc + 1) * CT]
                cw = VP(f"cw{mi}{cc}")
                srcr = pzv.rearrange("p (q t) -> p q t", t=W)
                accr = accv.rearrange("p (q t) -> p q t", t=W)
                nc.vector.tensor_scalar(accv, pzv, cw[:, 3:4], None,
                                        OP.mult)
                for k in range(3):
                    sh = 3 - k
                    nc.vector.scalar_tensor_tensor(
                        accr[:, :, sh:W], srcr[:, :, 0:W - sh],
                        cw[:, k:k + 1], accr[:, :, sh:W],
                        OP.mult, OP.add)
            for cc in range(2):
                bias = (None if flags["convb_zero"]
                        else VP(f"cb{mi}{cc}"))
                if bias is None:
                    nc.scalar.activation(xcb[cc][:, c0:c0 + CT],
                                         acc2[:, cc * CT:(cc + 1) * CT],
                                         AF.Silu)
                else:
                    nc.scalar.activation(xcb[cc][:, c0:c0 + CT],
                                         acc2[:, cc * CT:(cc + 1) * CT],
                                         AF.Silu, bias=bias)
            pbc = psC("pbc")
            for cc in range(2):
                mm(pbc[:40, :], BR(f"bc{mi}{cc}"),
                   xcb[cc][:, c0:c0 + CT], cc == 0, cc == 1)
            bcs = rg.tile([40, CT], bf16, name="bcs", tag="bcs", bufs=2)
            nc.scalar.activation(bcs[:], pbc[:40, :], AF.Copy)
            nc.sync.dma_start(aps[f"scr_bc_{bname}"][:, c0:c0 + CT],
                              bcs[:])
            yield

    # ============ dt factory: one tile -> dts ring slot ==============
    def emit_factory(bname, mi, ci, dts_ring):
        xcb, _, _ = tiles[bname]
        c0 = ci * CT
        dtin = rg.tile([8, CT], bf16, name="dtin", tag="dtin", bufs=2)
        nc.sync.dma_start(dtin[:],
                          aps[f"scr_bc_{bname}"][0:8, c0:c0 + CT])
        pd = psA("pd")
        for cc in range(2):
            mm(pd[:, cc * CT:(cc + 1) * CT], BR(f"dtw{mi}{cc}"),
               dtin[:], True, True, kp=8)
        ez = rg.tile([128, 2 * CT], bf16, name="ez", tag="ez", bufs=1)
        if flags["dtb_const"] is not None:
            nc.scalar.activation(ez[:], pd[:], AF.Exp, bias=VP("dtbc"))
        else:
            for cc in range(2):
                nc.scalar.activation(
                    ez[:, cc * CT:(cc + 1) * CT],
                    pd[:, cc * CT:(cc + 1) * CT],
                    AF.Exp, bias=VP(f"dtb{mi}{cc}"))
        dts = rg.tile([128, 2 * CT], bf16, name="dts", tag="dts", bufs=3)
        nc.scalar.activation(dts[:], ez[:], AF.Ln, bias=1.0)
        dtx = rg.tile([128, 2 * CT], bf16, name="dtx", tag="dtx", bufs=2)
        for cc in range(2):
            nc.vector.tensor_mul(dtx[:, cc * CT:(cc + 1) * CT],
                                 dts[:, cc * CT:(cc + 1) * CT],
                                 xcb[cc][:, c0:c0 + CT])
        for cc in range(2):
            nc.sync.dma_start(
                aps[f"scr_dtx_{bname}"][cc, ci]
                .rearrange("p (g c) -> g p c", g=16),
                dtx[:, cc * CT:(cc + 1) * CT])
        dtv = dts[:].rearrange("p (x t) -> p x t", t=W)
        nc.vector.tensor_scalar(dtv[:, :, 0:1], dtv[:, :, 0:1],
                                0.0, 1.0e4, OP.mult, OP.add)
        dts_ring[ci] = dts

    # ============ scan unit (one tile) ===============================
    def emit_scan(bname, mi, ci, dts_ring, is_t, s_tile, s_c0):
        xcb, zsb, xT = tiles[bname]
        c0 = ci * CT
        dts = dts_ring.pop(ci)
        Brep = rg.tile([128, CT], bf16, name="Brep", tag="Brep", bufs=2)
        nc.sync.dma_start(
            Brep[:],
            aps[f"scr_bc_{bname}"][8:24, c0:c0 + CT]
            .unsqueeze(0).broadcast_to((8, 16, CT)))
        Crep = rg.tile([128, CT], bf16, name="Crep", tag="Crep", bufs=2)
        nc.sync.dma_start(
            Crep[:],
            aps[f"scr_bc_{bname}"][24:40, c0:c0 + CT]
            .unsqueeze(0).broadcast_to((8, 16, CT)))
        ues = []
        for cc in range(2):
            halves = []
            for hf in range(2):
                ueh = rg.tile([128, 8 * CT], bf16, name="ueh",
                              tag=f"ueh{hf}", bufs=2 - hf)
                nc.sync.dma_start(
                    ueh[:],
                    aps[f"scr_dtx_{bname}"]
                    [cc, ci, :, hf * 8 * CT:(hf + 1) * 8 * CT]
                    .unsqueeze(1).broadcast_to((8, 16, 8 * CT)))
                halves.append(ueh)
            ues.append(halves)
        Brep_b = Brep[:].unsqueeze(1).broadcast_to((128, 2, CT))
        Crep_b = Crep[:].unsqueeze(1).broadcast_to((128, 2, CT))
        gzs = []
        for cc in range(2):
            pY = psY(f"pY{cc}")
            stage = []
            for gp in range(11):
                if gp < 8:
                    pP = psA("pP")
                    for i in range(2):
                        g = gp * 2 + i
                        mm(pP[:, i * CT:(i + 1) * CT],
                           dAW(mi, cc, g),
                           dts[:, cc * CT:(cc + 1) * CT],
                           True, True)
                    dA = rg.tile([128, 2 * CT], bf16, name="dA",
                                 tag="dA", bufs=3)
                    nc.scalar.activation(dA[:], pP[:], AF.Exp)
                    u2 = rg.tile([128, 2 * CT], bf16, name="u2",
                                 tag="u2", bufs=3)
                    ueh = ues[cc][gp // 4]
                    sl = (gp % 4) * 2 * CT
                    eng = nc.gpsimd if gp % 4 == 3 else nc.vector
                    eng.tensor_mul(
                        u2[:].rearrange("p (i c) -> p i c", i=2),
                        ueh[:, sl:sl + 2 * CT]
                        .rearrange("p (i c) -> p i c", i=2),
                        Brep_b)
                    stage.append((gp, dA, u2))
                if gp >= 3:
                    gq, dAq, u2q = stage.pop(0)
                    h2 = rg.tile([128, 2 * CT], bf16, name="h2",
                                 tag="h2", bufs=2)
                    for i in range(2):
                        nc.vector.tensor_tensor_scan(
                            h2[:, i * CT:(i + 1) * CT],
                            dAq[:, i * CT:(i + 1) * CT],
                            u2q[:, i * CT:(i + 1) * CT],
                            0.0, OP.mult, OP.add)
                    yh2 = rg.tile([128, 2 * CT], bf16, name="yh2",
                                  tag="yh2", bufs=2)
                    eng = nc.gpsimd if gq % 2 == 1 else nc.vector
                    eng.tensor_mul(
                        yh2[:].rearrange("p (i c) -> p i c", i=2),
                        h2[:].rearrange("p (i c) -> p i c", i=2),
                        Crep_b)
                    for i in range(2):
                        g = gq * 2 + i
                        mm(pY[:], BR(f"sum{g}"),
                           yh2[:, i * CT:(i + 1) * CT],
                           g == 0, g == 15)
            yg = rg.tile([128, CT], bf16, name=f"yg{cc}", tag=f"yg{cc}",
                         bufs=1)
            if flags["D_ones"]:
                nc.vector.tensor_add(yg[:], pY[:],
                                     xcb[cc][:, c0:c0 + CT])
            else:
                nc.vector.scalar_tensor_tensor(
                    yg[:], xcb[cc][:, c0:c0 + CT],
                    VP(f"Dp{mi}{cc}"), pY[:], OP.mult, OP.add)
            gz = rg.tile([128, CT], bf16, name=f"gz{cc}", tag=f"gz{cc}",
                         bufs=1)
            nc.gpsimd.tensor_mul(gz[:], yg[:], zsb[cc][:, c0:c0 + CT])
            gzs.append(gz)
        po = psC("po")
        for cc in range(2):
            mm(po[:], BR(f"wout{mi}{cc}"), gzs[cc][:], cc == 0, cc == 1)
        res = s_tile[:, s_c0:s_c0 + CT]
        if is_t:
            nc.vector.tensor_add(res, po[:], xT[:, c0:c0 + CT])
            s2 = rg.tile([128, CT], bf16, name="s2", tag="s2", bufs=1)
            nc.gpsimd.tensor_mul(s2[:], res, res)
            ln_stats(res, s2[:], (bname, "a"), c0)
            ln_finish((bname, "a"), c0)
        else:
            # n/l: write pre-mix value (ym) into the s tile; mix rewrites
            nc.scalar.activation(res, po[:], AF.Copy)

    # ========= factory + scan generator =====================
    # units: fac(0), fac(1), [scan(0), fac(2)], [scan(1), fac(3)], ...
    def g_fs(bname):
        _, mi, _, _, _ = HB[bname]
        is_t = bname[0] == "t"
        s_tile = s_full.get(bname)
        dts_ring = {}
        emit_factory(bname, mi, 0, dts_ring)
        yield
        emit_factory(bname, mi, 1, dts_ring)
        yield
        for ci in range(NT):
            if s_tile is not None:
                st, sc0 = s_tile, ci * CT
            else:
                st = rg.tile([128, CT], bf16, name="sT", tag="sT", bufs=8)
                tiles_s[(bname, ci)] = st
                sc0 = 0
            emit_scan(bname, mi, ci, dts_ring, is_t, st, sc0)
            if ci + 2 < NT:
                emit_factory(bname, mi, ci + 2, dts_ring)
            yield

    tiles_s = {}

    # ================= mix generator (8 units) =======================
    def g_mix(pair):
        nb, lb = pair
        q_n = HB[nb][3]
        q_l = HB[lb][3]
        for c0 in range(0, CBLK, CT):
            cat = {"n": s_full[nb][:, c0:c0 + CT],
                   "l": s_full[lb][:, c0:c0 + CT]}
            # both mix matmuls first: they read cat slices that the res
            # writes below overwrite in place (s tile doubles as ym)
            mss = []
            for mc in range(2):
                pmx = psC("pmx")
                for kc, kk in enumerate(("n", "l")):
                    mm(pmx[:], BR(f"mix{kc}{mc}"), cat[kk], kc == 0,
                       kc == 1)
                ms = rg.tile([128, CT], bf16, name="ms", tag=f"ms{mc}",
                             bufs=1)
                if flags["mixb_zero"]:
                    nc.scalar.activation(ms[:], pmx[:], AF.Silu)
                else:
                    nc.scalar.activation(ms[:], pmx[:], AF.Silu,
                                         bias=VP(f"mixb{mc}"))
                mss.append(ms)
            # t2 adds also read cat before any res write
            t2s = []
            for mc, key in enumerate(("n", "l")):
                t2 = rg.tile([128, CT], bf16, name="t2", tag=f"t2{mc}",
                             bufs=1)
                nc.vector.tensor_add(t2[:], cat[key], mss[mc][:])
                t2s.append(t2)
            for mc, (key, bn, xk, qo) in enumerate(
                    (("n", nb, "x_n", q_n), ("l", lb, "x_l", q_l))):
                q0 = qo + c0 // W
                raw = rg.tile([64, 8 * 128], f32, name="rawm", tag="raw",
                              bufs=2)
                nc.sync.dma_start(raw[:], aps[xk][:, q0:q0 + 8, :])
                ptx = psC("ptx")
                for i in range(8):
                    nc.tensor.transpose(ptx[:, i * 64:(i + 1) * 64],
                                        raw[:, i * 128:(i + 1) * 128],
                                        I64[:64, :64])
                res = s_full[bn][:, c0:c0 + CT]
                nc.vector.tensor_add(res, t2s[mc][:], ptx[:])
                s2m = rg.tile([128, CT], bf16, name="s2m", tag="s2m",
                              bufs=2)
                nc.gpsimd.tensor_mul(s2m[:], res, res)
                ln_stats(res, s2m[:], (bn, "a"), c0)
            yield

    # ================= phase C generator (16 units) ==================
    def g_C(bname, fin_a):
        _, mi, _, _, j0 = HB[bname]
        is_t = bname[0] == "t"

        def CF(ci):
            c0 = ci * CT
            if is_t:
                src = tiles_s.pop((bname, ci))[:]
            else:
                src = s_full[bname][:, c0:c0 + CT]
            n1 = rg.tile([128, CT], bf16, name="n1", tag="n1", bufs=2)
            ga, gb = (None, None) if flags["an_id"] else \
                (f"ang{mi}", f"anb{mi}")
            ln_apply(src, (bname, "a"), c0, ga, gb, n1[:])
            hh = rg.tile([128, 4 * CT], bf16, name="hh", tag="hh", bufs=1)
            for hp in range(2):
                pf = psA("pf")
                for ci2 in range(2):
                    c4 = hp * 2 + ci2
                    mm(pf[:, ci2 * CT:(ci2 + 1) * CT],
                       BR(f"ff1{mi}")[:, c4 * 128:(c4 + 1) * 128],
                       n1[:], True, True)
                if flags["f1b_zero"]:
                    nc.scalar.activation(
                        hh[:, hp * 2 * CT:(hp + 1) * 2 * CT], pf[:],
                        AF.Prelu, alpha=0.01)
                else:
                    for ci2 in range(2):
                        c4 = hp * 2 + ci2
                        nc.scalar.activation(
                            hh[:, c4 * CT:(c4 + 1) * CT],
                            pf[:, ci2 * CT:(ci2 + 1) * CT], AF.Prelu,
                            bias=VP(f"f1b{mi}{c4}"), alpha=0.01)
            pf2 = psC("pf2")
            for c4 in range(4):
                mm(pf2[:], BR(f"ff2{mi}{c4}"),
                   hh[:, c4 * CT:(c4 + 1) * CT], c4 == 0, c4 == 3)
            sf = rg.tile([128, CT], bf16, name="sf", tag="sf", bufs=2)
            if flags["f2b_zero"]:
                nc.vector.tensor_add(sf[:], pf2[:], n1[:])
            else:
                nc.vector.scalar_tensor_tensor(sf[:], pf2[:],
                                               VP(f"f2b{mi}"),
                                               n1[:], OP.add, OP.add)
            s2f = rg.tile([128, CT], bf16, name="s2f", tag="s2f", bufs=1)
            nc.gpsimd.tensor_mul(s2f[:], sf[:], sf[:])
            ln_stats(sf[:], s2f[:], (bname, "f"), c0)
            return sf

        def CT_out(ci, sf):
            c0 = ci * CT
            ga, gb = (None, None) if flags["fln_id"] else \
                (f"flg{mi}", f"flb{mi}")
            n2 = rg.tile([128, CT], f32, name="n2", tag="n2", bufs=1)
            ln_apply(sf[:], (bname, "f"), c0, ga, gb, n2[:])
            pto = psC("pto")
            for c in range(4):
                nc.tensor.transpose(pto[:, c * 128:(c + 1) * 128],
                                    n2[:, c * 128:(c + 1) * 128],
                                    I128)
            ot = rg.tile([128, CT], f32, name="ot", tag="ot", bufs=1)
            nc.vector.tensor_copy(ot[:], pto[:])
            q0 = c0 // W
            for qh in range(2):
                nc.sync.dma_start(
                    aps["out"][:, j0 + q0:j0 + q0 + 8, :]
                    .rearrange("t (c q) d -> q t c d", c=4)[qh],
                    ot[qh * 64:(qh + 1) * 64, :]
                    .rearrange("t (c d) -> t c d", c=4))

        # software-pipelined: finishes run one tile ahead of the applies
        # that consume their DRAM-bounced scale rows, so the rnm2 load DMA
        # never holds the DMA queue waiting on a just-issued write.
        if fin_a:
            ln_finish((bname, "a"), 0)
        sf_prev = None
        for ci in range(NT):
            if is_t:
                while (bname, ci) not in tiles_s:
                    yield   # scan hasn't produced this tile yet; spin
            if fin_a and ci + 1 < NT:
                ln_finish((bname, "a"), (ci + 1) * CT)
            sf = CF(ci)
            if ci >= 1:
                ln_finish((bname, "f"), (ci - 1) * CT)
            yield
            if sf_prev is not None:
                CT_out(ci - 1, sf_prev)
                yield
            sf_prev = sf
        ln_finish((bname, "f"), (NT - 1) * CT)
        yield
        CT_out(NT - 1, sf_prev)
        yield

    # ===================== master schedule ===========================
    with tc.tile_pool(name="a_sq", bufs=1) as a_sqp:
        cq = []   # persistent queue of pending C generators

        def A1(bn):
            return g_a1(bn, a_sqp)

        # order: n0, l0, t0, n1, l1, t1, t2, t3 — a family's C phase fully
        # drains inside t-block windows before its s/sT tag is reused.
        _drain(A1("n0"))
        _drain(g_fs("n0"))
        _drain(A1("l0"))
        _drain(g_fs("l0"))
        _weave([(A1("t0"), 1), (g_mix(("n0", "l0")), 1)])
        C_n0 = GStream(g_C("n0", True))
        C_l0 = GStream(g_C("l0", True))
        C_t0 = GStream(_delay(5, g_C("t0", False)))
        cq.extend([C_n0, C_l0, C_t0])
        _weave_until(g_fs("t0"), cq, w_master=1, w_c=3)
        C_n0.finish()          # s_n tag is rewritten by scan(n1)
        _drain(A1("n1"))       # silu window: no C (act-table isolation)
        _weave_until(g_fs("n1"), cq, w_master=1, w_c=3)
        C_l0.finish()          # s_l tag is rewritten by scan(l1)
        _drain(A1("l1"))
        _weave_until(g_fs("l1"), cq, w_master=1, w_c=3)
        _weave([(A1("t1"), 1), (g_mix(("n1", "l1")), 1)])
        C_prev = C_t0
        cq.extend([GStream(g_C("n1", True)), GStream(g_C("l1", True))])
        for bn, nxt in (("t1", "t2"), ("t2", "t3"), ("t3", None)):
            C_prev.finish()    # sT ring slots reused by scan(bn)
            C_cur = GStream(_delay(5, g_C(bn, False)))
            cq.append(C_cur)
            _weave_until(g_fs(bn), cq, w_master=1, w_c=4)
            if nxt is not None:
                _drain(A1(nxt))
            C_prev = C_cur
        for g in cq:
            g.finish()


def _build_program(wp, vp, bpk, flags):
    nc = bacc.Bacc("TRN2", target_bir_lowering=False, debug=False,
                   num_devices=N_CORES)
    aps = {}
    aps["x_n"] = nc.dram_tensor("x_n", [W, N, D], f32,
                                kind="ExternalInput").ap()
    aps["x_t"] = nc.dram_tensor("x_t", [W, E, D], f32,
                                kind="ExternalInput").ap()
    aps["x_l"] = nc.dram_tensor("x_l", [W, N, D], f32,
                                kind="ExternalInput").ap()
    aps["wpack"] = nc.dram_tensor("wpack", [128, wp.n], f32,
                                  kind="ExternalInput").ap()
    aps["vpack"] = nc.dram_tensor("vpack", [128, vp.n], f32,
                                  kind="ExternalInput").ap()
    aps["bpack"] = nc.dram_tensor("bpack", [128, bpk.n], bf16,
                                  kind="ExternalInput").ap()
    aps["out"] = nc.dram_tensor("out", [W, 2 * N + E, D], f32,
                                kind="ExternalOutput").ap()
    for bname, _, _, _, _ in HBLOCKS:
        aps[f"scr_bc_{bname}"] = nc.dram_tensor(
            f"scr_bc_{bname}", [40, CBLK], bf16).ap()
        aps[f"scr_dtx_{bname}"] = nc.dram_tensor(
            f"scr_dtx_{bname}", [2, NT, 8, 16 * CT], bf16).ap()
        for ph in ("a", "f"):
            aps[f"scr_rn_{bname}_{ph}"] = nc.dram_tensor(
                f"scr_rn_{bname}_{ph}", [2, 128, NW], bf16).ap()

    with tile.TileContext(nc) as tc:
        with ExitStack() as ctx:
            _emit(ctx, tc, nc, aps, wp, vp, bpk, flags)
    nc.compile()
    return nc


_CACHE = {}


def kernel(**inputs):
    wp, vp, bpk, flags = _host_pack(inputs)
    if "prog" not in _CACHE:
        _CACHE["prog"] = _build_program(wp, vp, bpk, flags)
    nc = _CACHE["prog"]
    wpack, vpack = wp.build(), vp.build()
    bpack = bpk.build().astype(ml_dtypes.bfloat16)
    in_maps = []
    for b in range(B):
        in_maps.append({
            "x_n": np.ascontiguousarray(inputs["x_node"][b]),
            "x_t": np.ascontiguousarray(inputs["x_trace"][b]),
            "x_l": np.ascontiguousarray(inputs["x_log"][b]),
            "wpack": wpack,
            "vpack": vpack,
            "bpack": bpack,
        })
    res = run_bass_kernel_spmd(nc, in_maps, list(range(N_CORES)))
    out = np.stack([res.results[b]["out"] for b in range(B)], axis=0)
    return out.astype(np.float32)


# revision 26
# speedup vs baseline: 1.1345x; 1.0000x over previous
"""Trainium2 Bass kernel for nn_Encoder (tri-modal Mamba encoder), v3.

kernel(**inputs) takes FULL unsharded numpy inputs and returns the FULL
output (B, W, 2N+E, D). Batch B=8 is sharded across 8 NeuronCores (pure
data parallel, no collectives); params are replicated.

v3 vs v2 (same math, new orchestration):
- Half-block (64-seq) processing units, software-pipelined end to end:
  the AddNorm+FFN+output phase (C) of each half-block is emitted as a
  persistent generator that drains into whatever later window has engine
  slack, so the serial phase-C tail is gone.
- LayerNorm stats finish per column-tile with rsqrt = exp(-0.5*ln(v+eps))
  so interleaved phases stay inside the natural_log_exp activation-table
  set (Silu windows are kept separate: A1 and mix).
- s / LN stats / LN scale factors live in SBUF; ym merged into s tiles.
- One shared 8-bank PSUM tag discipline (psA 2x[128,1024]f32,
  psY 2x[128,512], psC 2x[128,512]) across all phases.
- Output path: 4 batched f32 transposes per 512-col tile, one copy, one
  DMA (q t d scatter) straight from the LN-f apply.
"""

import functools

import ml_dtypes
import numpy as np
from contextlib import ExitStack

import concourse.bass as bass
import concourse.tile as tile
import concourse.bacc as bacc_mod
import concourse.hw_specs as hw_specs_mod
from concourse import bacc, mybir
from concourse.bass_utils import run_bass_kernel_spmd

# Prefer the exp+ln combined activation-table set so the Exp/Ln mix
# (softplus factory, scan decay, LN rsqrt) resolves to ONE table and the
# act-table load pass stops thrashing between exp_and_others/natural_log.
_GAT_ORIG = hw_specs_mod.get_activation_tables.__wrapped__


@functools.cache
def _gat_reordered(arch):
    t = dict(_GAT_ORIG(arch))
    pref = [k for k in ("natural_log_exp_and_others",) if k in t]
    return {k: t[k] for k in pref + [k for k in t if k not in pref]}


hw_specs_mod.get_activation_tables = _gat_reordered
bacc_mod.get_activation_tables = _gat_reordered

D, DI, SS, KK, RR = 128, 256, 16, 4, 8
B, W, N, E = 8, 64, 128, 256
Q = 64                       # seqs per half-block
CBLK = Q * W                 # 4096 cols per half-block
CT = 512                     # column tile (8 seqs)
NT = CBLK // CT              # 8 tiles per half-block
NW = CBLK // 128             # 32 stat cols per partition
f32 = mybir.dt.float32
f32r = mybir.dt.float32r
bf16 = mybir.dt.bfloat16
AF = mybir.ActivationFunctionType
OP = mybir.AluOpType

# (name, modality, input key, q offset, output entity offset)
HBLOCKS = [("n0", 0, "x_n", 0, 0), ("n1", 0, "x_n", 64, 64),
           ("l0", 2, "x_l", 0, 384), ("l1", 2, "x_l", 64, 448),
           ("t0", 1, "x_t", 0, 128), ("t1", 1, "x_t", 64, 192),
           ("t2", 1, "x_t", 128, 256), ("t3", 1, "x_t", 192, 320)]
HB = {b[0]: b for b in HBLOCKS}
N_CORES = 8
LN_EPS = 1e-5


class Pack:
    def __init__(self):
        self.cols = []
        self.off = {}
        self.n = 0

    def add(self, name, arr, dtype=np.float32):
        arr = np.asarray(arr, dtype)
        assert arr.ndim == 2 and arr.shape[0] <= 128
        a = np.zeros((128, arr.shape[1]), dtype)
        a[: arr.shape[0]] = arr
        self.off[name] = (self.n, arr.shape[1])
        self.cols.append(a)
        self.n += arr.shape[1]

    def build(self):
        return np.concatenate(self.cols, axis=1)


def _host_pack(inp):
    """Returns (wp f32-staged-to-f32r, vp f32, bp bf16, flags)."""
    flags = {}
    dtb = np.asarray(inp["mp_dt_b"], np.float64)
    flags["dtb_const"] = float(dtb.flat[0]) if np.ptp(dtb) < 1e-12 else None
    flags["D_ones"] = bool(np.allclose(np.asarray(inp["mp_D"]), 1.0))
    flags["convb_zero"] = bool(np.all(np.asarray(inp["mp_conv_b"]) == 0.0))
    flags["f1b_zero"] = bool(np.all(np.asarray(inp["ff1_b"]) == 0.0))
    flags["f2b_zero"] = bool(np.all(np.asarray(inp["ff2_b"]) == 0.0))
    flags["an_id"] = bool(np.all(np.asarray(inp["an_g"]) == 1.0)
                          and np.all(np.asarray(inp["an_b"]) == 0.0))
    flags["fln_id"] = bool(np.all(np.asarray(inp["fln_g"]) == 1.0)
                           and np.all(np.asarray(inp["fln_b"]) == 0.0))
    flags["mixb_zero"] = bool(np.all(np.asarray(inp["mix_b"]) == 0.0))
    A = -np.exp(np.asarray(inp["mp_Alog"], np.float64))      # (3, DI, S)
    flags["A_shared"] = bool(
        np.ptp(A, axis=(0, 1)).max() < 1e-9 * np.abs(A).max())

    bp = Pack()   # bf16 weights
    for g in range(16):
        sm = np.zeros((128, 128), np.float32)
        for k in range(128):
            sm[k, g * 8 + k // 16] = 1.0
        bp.add(f"sum{g}", sm)

    def delta_A(Am, cc, g):
        dl = np.zeros((128, 128), np.float32)
        for j in range(128):
            dl[g * 8 + j // 16, j] = Am[cc * 128 + g * 8 + j // 16, j % 16]
        return dl

    if flags["A_shared"]:
        for g in range(16):
            bp.add(f"dA{g}", delta_A(A[0], 0, g))
    else:
        for m in range(3):
            for cc in range(2):
                for g in range(16):
                    bp.add(f"dA{m}{cc}{g}", delta_A(A[m], cc, g))

    wp = Pack()   # fp32 staged -> f32r on device
    vp = Pack()   # fp32 per-partition vectors
    for m in range(3):
        bp.add(f"win{m}", inp["mp_in"][m])                   # (D, 512)
        wxp = inp["mp_xproj"][m]                             # (DI, 40)
        for cc in range(2):
            bp.add(f"bc{m}{cc}", wxp[cc * 128:(cc + 1) * 128])
        dtw = inp["mp_dt_w"][m]                              # (R, DI)
        for cc in range(2):
            bp.add(f"dtw{m}{cc}", dtw[:, cc * 128:(cc + 1) * 128])
        wout = inp["mp_out"][m]                              # (DI, D)
        for cc in range(2):
            bp.add(f"wout{m}{cc}", wout[cc * 128:(cc + 1) * 128])
        bp.add(f"ff1{m}", inp["ff1_w"][m])                   # (D, 512)
        ff2 = inp["ff2_w"][m]                                # (4D, D)
        for c4 in range(4):
            bp.add(f"ff2{m}{c4}", ff2[c4 * 128:(c4 + 1) * 128])
    mixw = inp["mix_w"]
    for kc in range(2):
        for mc in range(2):
            bp.add(f"mix{kc}{mc}", mixw[kc * 128:(kc + 1) * 128,
                                        mc * 128:(mc + 1) * 128])
    wp.add("onesD", np.full((128, 1), 1.0 / D, np.float32))
    bp.add("onesDb", np.full((128, 1), 1.0 / D, np.float32))
    wp.add("ones1", np.ones((1, 128), np.float32))

    vp.add("eps", np.full((128, 1), LN_EPS, np.float32))
    if flags["dtb_const"] is not None:
        vp.add("dtbc", np.full((128, 1), flags["dtb_const"], np.float32))
    vp.add("I64", np.eye(64, dtype=np.float32))
    vp.add("I128", np.eye(128, dtype=np.float32))
    for m in range(3):
        cw = inp["mp_conv_w"][m]
        for cc in range(2):
            sl = slice(cc * 128, (cc + 1) * 128)
            vp.add(f"cw{m}{cc}", cw[sl])                     # 4 cols
            if not flags["convb_zero"]:
                vp.add(f"cb{m}{cc}", inp["mp_conv_b"][m][sl, None])
            if flags["dtb_const"] is None:
                vp.add(f"dtb{m}{cc}", inp["mp_dt_b"][m][sl, None])
            if not flags["D_ones"]:
                vp.add(f"Dp{m}{cc}", inp["mp_D"][m][sl, None])
        if not flags["f1b_zero"]:
            for c4 in range(4):
                vp.add(f"f1b{m}{c4}",
                       inp["ff1_b"][m][c4 * 128:(c4 + 1) * 128, None])
        if not flags["f2b_zero"]:
            vp.add(f"f2b{m}", inp["ff2_b"][m][:, None])
        if not flags["an_id"]:
            vp.add(f"ang{m}", inp["an_g"][m][:, None])
            vp.add(f"anb{m}", inp["an_b"][m][:, None])
        if not flags["fln_id"]:
            vp.add(f"flg{m}", inp["fln_g"][m][:, None])
            vp.add(f"flb{m}", inp["fln_b"][m][:, None])
    if not flags["mixb_zero"]:
        for mc in range(2):
            vp.add(f"mixb{mc}", inp["mix_b"][mc * 128:(mc + 1) * 128, None])
    return wp, vp, bp, flags


def _drain(g):
    for _ in g:
        pass


def _weave(streams):
    """streams: (gen, weight) pairs; round-robin to exhaustion."""
    live = [[iter(g), w] for g, w in streams]
    while live:
        for ent in list(live):
            g, w = ent
            for _ in range(w):
                try:
                    next(g)
                except StopIteration:
                    live.remove(ent)
                    break


class GStream:
    def __init__(self, g):
        self.g = iter(g)
        self.done = False

    def step(self):
        if self.done:
            return False
        try:
            next(self.g)
            return True
        except StopIteration:
            self.done = True
            return False

    def finish(self):
        while self.step():
            pass


def _weave_until(master, cs, w_master=1, w_c=1):
    """Interleave master with the persistent queue `cs` (GStream list).
    Returns when master is exhausted; cs keeps its remaining state."""
    m = iter(master)
    while True:
        for _ in range(w_master):
            try:
                next(m)
            except StopIteration:
                return
        budget = w_c
        while budget > 0 and cs:
            if cs[0].step():
                budget -= 1
            else:
                cs.pop(0)


def _delay(n, g):
    for _ in range(n):
        yield
    yield from g


def _emit(ctx, tc, nc, aps, wp, vp, bpk, flags):
    wpool = ctx.enter_context(tc.tile_pool(name="weights", bufs=1))
    wr = wpool.tile([128, wp.n], f32r, name="wr", tag="wr")
    vec = wpool.tile([128, vp.n], f32, name="vec", tag="vec")
    nc.sync.dma_start(vec[:], aps["vpack"][:])
    bw = wpool.tile([128, bpk.n], bf16, name="bw", tag="bw")
    nc.sync.dma_start(bw[:], aps["bpack"][:])
    with tc.tile_pool(name="wstage", bufs=1) as stpool:
        wstage = stpool.tile([128, wp.n], f32, name="wstage")
        nc.sync.dma_start(wstage[:], aps["wpack"][:])
        for o in range(0, wp.n, 8192):
            e = min(wp.n, o + 8192)
            nc.vector.tensor_copy(wr[:, o:e], wstage[:, o:e])

    def WR(name):
        o, c = wp.off[name]
        return wr[:, o:o + c]

    def VP(name):
        o, c = vp.off[name]
        return vec[:, o:o + c]

    def BR(name):
        o, c = bpk.off[name]
        return bw[:, o:o + c]

    def mm(psum_ap, lhsT_ap, rhs_ap, start, stop, kp=128):
        nc.tensor.matmul(psum_ap, lhsT_ap[:kp, :], rhs_ap[:kp, :],
                         start=start, stop=stop)

    def dAW(mi, cc, g):
        return BR(f"dA{g}" if flags["A_shared"] else f"dA{mi}{cc}{g}")

    I64 = VP("I64")
    I128 = VP("I128")

    # ---- long-lived SBUF state --------------------------------------
    blk = ctx.enter_context(tc.tile_pool(name="blk", bufs=1))
    # s tiles: n/l halves need full (128, CBLK); mix writes them in place
    # over the ym values (same storage). t halves use a small ring.
    s_full = {nm: blk.tile([128, CBLK], bf16, name=f"s_{nm}",
                           tag=f"s_{nm[0]}")
              for nm in ("n0", "n1", "l0", "l1")}  # tag per family: 2 tags
    statp = ctx.enter_context(tc.tile_pool(name="stat", bufs=1))
    stat, rnm = {}, {}
    for bname, _, _, _, _ in HBLOCKS:
        for ph in ("a", "f"):
            stat[(bname, ph)] = statp.tile(
                [128, 2 * NW], f32, name=f"st_{bname}{ph}",
                tag=f"st_{bname}{ph}")
            rnm[(bname, ph)] = statp.tile(
                [128, 2 * NW], bf16, name=f"rn_{bname}{ph}",
                tag=f"rn_{bname}{ph}")

    # ---- shared PSUM tags (8 banks total) ---------------------------
    ps = ctx.enter_context(tc.tile_pool(name="ps", bufs=1, space="PSUM"))

    def psA(name):
        return ps.tile([128, 2 * CT], f32, name=name, tag="psA", bufs=2)

    def psS(name):
        return ps.tile([1, 2 * CT], f32, name=name, tag="psA", bufs=2)

    def psY(name):
        return ps.tile([128, CT], f32, name=name, tag="psY", bufs=2)

    def psC(name):
        return ps.tile([128, CT], f32, name=name, tag="psC", bufs=2)

    rg = ctx.enter_context(tc.tile_pool(name="rg", bufs=1))

    # ---- LN helpers -------------------------------------------------
    def ln_stats(src_ap, sq_ap, statT, c0):
        ob = BR("onesDb")[:, 0:1]
        of = WR("onesD")[:, 0:1]
        pmq = psS("pmq")
        mm(pmq[:, 0:CT], ob if src_ap.dtype == bf16 else of, src_ap,
           True, True)
        mm(pmq[:, CT:2 * CT], ob if sq_ap.dtype == bf16 else of, sq_ap,
           True, True)
        sst = rg.tile([1, 2 * CT], f32, name="sst", tag="sst", bufs=1)
        nc.scalar.activation(sst[:], pmq[:], AF.Copy)
        p0 = (c0 // CT) * 16
        nc.sync.dma_start(
            stat[statT][p0:p0 + 16, :].rearrange("p (h w) -> h p w", h=2),
            sst[:].rearrange("x (h p w) -> x h p w", h=2, p=16))

    def ln_finish(statT, c0):
        """Finish LN scale factors for one column tile (16 partitions)."""
        p0 = (c0 // CT) * 16
        sT = stat[statT]
        rT = rnm[statT]
        m_t = sT[p0:p0 + 16, 0:NW]
        q_t = sT[p0:p0 + 16, NW:2 * NW]
        var = rg.tile([16, NW], f32, name="var", tag="lnvar", bufs=2)
        nc.vector.tensor_mul(var[:], m_t, m_t)
        nc.vector.tensor_sub(var[:], q_t, var[:])
        lnv = rg.tile([16, NW], f32, name="lnv", tag="lnlnv", bufs=2)
        nc.scalar.activation(lnv[:], var[:], AF.Ln, bias=VP("eps")[0:16])
        r_t = rg.tile([16, NW], f32, name="lnr", tag="lnr", bufs=2)
        nc.scalar.activation(r_t[:], lnv[:], AF.Exp, scale=-0.5)
        nmr = rg.tile([16, NW], f32, name="nmr", tag="lnnmr", bufs=2)
        nc.vector.tensor_mul(nmr[:], m_t, r_t[:])
        nc.vector.tensor_scalar(nmr[:], nmr[:], -1.0, None, OP.mult)
        nc.vector.tensor_copy(rT[p0:p0 + 16, 0:NW], r_t[:])
        nc.vector.tensor_copy(rT[p0:p0 + 16, NW:2 * NW], nmr[:])
        bn, ph = statT
        nc.sync.dma_start(
            aps[f"scr_rn_{bn}_{ph}"][:, p0:p0 + 16, :],
            rT[p0:p0 + 16, :].rearrange("p (h w) -> h p w", h=2))

    def ln_apply(src_ap, rnmT, c0, gk, bk, out_ap):
        p0 = (c0 // CT) * 16
        rnm2 = rg.tile([128, 2 * CT], bf16, name="rnm2", tag="rnm2",
                       bufs=2)
        bn, ph = rnmT
        nc.gpsimd.dma_start(
            rnm2[:].rearrange("j (h ab) -> j h ab", h=2),
            aps[f"scr_rn_{bn}_{ph}"][:, p0:p0 + 16, :]
            .rearrange("h p w -> h (p w)")
            .unsqueeze(0).broadcast_to((128, 2, CT)))
        t1 = rg.tile([128, CT], bf16, name="t1", tag="t1", bufs=2)
        nc.vector.tensor_mul(t1[:], src_ap, rnm2[:, 0:CT])
        if gk is None:
            nc.vector.tensor_add(out_ap, t1[:], rnm2[:, CT:2 * CT])
        else:
            nc.vector.tensor_add(t1[:], t1[:], rnm2[:, CT:2 * CT])
            nc.vector.tensor_scalar(out_ap, t1[:], VP(gk), VP(bk),
                                    OP.mult, OP.add)

    tiles = {}

    # ================= phase A1 (8 units) ============================
    def g_a1(bname, sqp):
        _, mi, xkey, q_off, _ = HB[bname]
        xcb = [sqp.tile([128, CBLK], bf16, name=f"xcb{cc}", tag=f"xcb{cc}")
               for cc in range(2)]
        zsb = [sqp.tile([128, CBLK], bf16, name=f"zsb{cc}", tag=f"zsb{cc}")
               for cc in range(2)]
        xT = sqp.tile([128, CBLK], bf16, name="xT", tag="xT")
        tiles[bname] = (xcb, zsb, xT)
        for c0 in range(0, CBLK, CT):
            q0 = c0 // W
            raw = rg.tile([64, 8 * 128], f32, name="raw", tag="raw", bufs=1)
            nc.sync.dma_start(raw[:],
                              aps[xkey][:, q_off + q0:q_off + q0 + 8, :])
            pt = psC("pt")
            for i in range(8):
                nc.tensor.transpose(pt[:, i * 64:(i + 1) * 64],
                                    raw[:, i * 128:(i + 1) * 128],
                                    I64[:64, :64])
            nc.scalar.activation(xT[:, c0:c0 + CT], pt[:], AF.Copy)
            xt_t = xT[:, c0:c0 + CT]
            pxc2 = psA("pxc2")
            for cc in range(2):
                mm(pxc2[:, cc * CT:(cc + 1) * CT],
                   BR(f"win{mi}")[:, cc * 128:(cc + 1) * 128], xt_t,
                   True, True)
            pz2 = psA("pz2")
            for cc in range(2):
                mm(pz2[:, cc * CT:(cc + 1) * CT],
                   BR(f"win{mi}")[:, (2 + cc) * 128:(3 + cc) * 128],
                   xt_t, True, True)
            for cc in range(2):
                nc.scalar.activation(zsb[cc][:, c0:c0 + CT],
                                     pz2[:, cc * CT:(cc + 1) * CT],
                                     AF.Silu)
            zc = rg.tile([128, 2 * CT], bf16, name="zc", tag="zc", bufs=1)
            nc.scalar.activation(zc[:], pxc2[:], AF.Copy)
            acc2 = rg.tile([128, 2 * CT], f32, name="acc2", tag="acc2",
                           bufs=1)
            for cc in range(2):
                pzv = zc[:, cc * CT:(cc + 1) * CT]
                accv = acc2[:, cc * CT:(cc + 1) * CT]
                cw = VP(f"cw{mi}{cc}")
                srcr = pzv.rearrange("p (q t) -> p q t", t=W)
                accr = accv.rearrange("p (q t) -> p q t", t=W)
                nc.vector.tensor_scalar(accv, pzv, cw[:, 3:4], None,
                                        OP.mult)
                for k in range(3):
                    sh = 3 - k
                    nc.vector.scalar_tensor_tensor(
                        accr[:, :, sh:W], srcr[:, :, 0:W - sh],
                        cw[:, k:k + 1], accr[:, :, sh:W],
                        OP.mult, OP.add)
            for cc in range(2):
                bias = (None if flags["convb_zero"]
                        else VP(f"cb{mi}{cc}"))
                if bias is None:
                    nc.scalar.activation(xcb[cc][:, c0:c0 + CT],
                                         acc2[:, cc * CT:(cc + 1) * CT],
                                         AF.Silu)
                else:
                    nc.scalar.activation(xcb[cc][:, c0:c0 + CT],
                                         acc2[:, cc * CT:(cc + 1) * CT],
                                         AF.Silu, bias=bias)
            pbc = psC("pbc")
            for cc in range(2):
                mm(pbc[:40, :], BR(f"bc{mi}{cc}"),
                   xcb[cc][:, c0:c0 + CT], cc == 0, cc == 1)
            bcs = rg.tile([40, CT], bf16, name="bcs", tag="bcs", bufs=2)
            nc.scalar.activation(bcs[:], pbc[:40, :], AF.Copy)
            nc.sync.dma_start(aps[f"scr_bc_{bname}"][:, c0:c0 + CT],
                              bcs[:])
            yield

    # ============ dt factory: one tile -> dts ring slot ==============
    def emit_factory(bname, mi, ci, dts_ring):
        xcb, _, _ = tiles[bname]
        c0 = ci * CT
        dtin = rg.tile([8, CT], bf16, name="dtin", tag="dtin", bufs=4)
        nc.sync.dma_start(dtin[:],
                          aps[f"scr_bc_{bname}"][0:8, c0:c0 + CT])
        pd = psA("pd")
        for cc in range(2):
            mm(pd[:, cc * CT:(cc + 1) * CT], BR(f"dtw{mi}{cc}"),
               dtin[:], True, True, kp=8)
        ez = rg.tile([128, 2 * CT], bf16, name="ez", tag="ez", bufs=1)
        if flags["dtb_const"] is not None:
            nc.scalar.activation(ez[:], pd[:], AF.Exp, bias=VP("dtbc"))
        else:
            for cc in range(2):
                nc.scalar.activation(
                    ez[:, cc * CT:(cc + 1) * CT],
                    pd[:, cc * CT:(cc + 1) * CT],
                    AF.Exp, bias=VP(f"dtb{mi}{cc}"))
        dts = rg.tile([128, 2 * CT], bf16, name="dts", tag="dts", bufs=3)
        nc.scalar.activation(dts[:], ez[:], AF.Ln, bias=1.0)
        dtx = rg.tile([128, 2 * CT], bf16, name="dtx", tag="dtx", bufs=2)
        for cc in range(2):
            nc.vector.tensor_mul(dtx[:, cc * CT:(cc + 1) * CT],
                                 dts[:, cc * CT:(cc + 1) * CT],
                                 xcb[cc][:, c0:c0 + CT])
        for cc in range(2):
            nc.sync.dma_start(
                aps[f"scr_dtx_{bname}"][cc, ci]
                .rearrange("p (g c) -> g p c", g=16),
                dtx[:, cc * CT:(cc + 1) * CT])
        dtv = dts[:].rearrange("p (x t) -> p x t", t=W)
        nc.vector.tensor_scalar(dtv[:, :, 0:1], dtv[:, :, 0:1],
                                0.0, 1.0e4, OP.mult, OP.add)
        dts_ring[ci] = dts

    # ============ scan unit (one tile) ===============================
    def emit_scan(bname, mi, ci, dts_ring, is_t, s_tile, s_c0):
        xcb, zsb, xT = tiles[bname]
        c0 = ci * CT
        dts = dts_ring.pop(ci)
        Brep = rg.tile([128, CT], bf16, name="Brep", tag="Brep", bufs=3)
        nc.sync.dma_start(
            Brep[:],
            aps[f"scr_bc_{bname}"][8:24, c0:c0 + CT]
            .unsqueeze(0).broadcast_to((8, 16, CT)))
        Crep = rg.tile([128, CT], bf16, name="Crep", tag="Crep", bufs=3)
        nc.sync.dma_start(
            Crep[:],
            aps[f"scr_bc_{bname}"][24:40, c0:c0 + CT]
            .unsqueeze(0).broadcast_to((8, 16, CT)))
        ues = []
        for cc in range(2):
            halves = []
            for hf in range(2):
                ueh = rg.tile([128, 8 * CT], bf16, name="ueh",
                              tag=f"ueh{hf}", bufs=2 - hf)
                nc.gpsimd.dma_start(
                    ueh[:],
                    aps[f"scr_dtx_{bname}"]
                    [cc, ci, :, hf * 8 * CT:(hf + 1) * 8 * CT]
                    .unsqueeze(1).broadcast_to((8, 16, 8 * CT)))
                halves.append(ueh)
            ues.append(halves)
        Brep_b = Brep[:].unsqueeze(1).broadcast_to((128, 2, CT))
        Crep_b = Crep[:].unsqueeze(1).broadcast_to((128, 2, CT))
        gzs = []
        for cc in range(2):
            pY = psY(f"pY{cc}")
            stage = []
            for gp in range(11):
                if gp < 8:
                    pP = psA("pP")
                    for i in range(2):
                        g = gp * 2 + i
                        mm(pP[:, i * CT:(i + 1) * CT],
                           dAW(mi, cc, g),
                           dts[:, cc * CT:(cc + 1) * CT],
                           True, True)
                    dA = rg.tile([128, 2 * CT], bf16, name="dA",
                                 tag="dA", bufs=3)
                    nc.scalar.activation(dA[:], pP[:], AF.Exp)
                    u2 = rg.tile([128, 2 * CT], bf16, name="u2",
                                 tag="u2", bufs=3)
                    ueh = ues[cc][gp // 4]
                    sl = (gp % 4) * 2 * CT
                    eng = nc.gpsimd if gp % 4 == 3 else nc.vector
                    eng.tensor_mul(
                        u2[:].rearrange("p (i c) -> p i c", i=2),
                        ueh[:, sl:sl + 2 * CT]
                        .rearrange("p (i c) -> p i c", i=2),
                        Brep_b)
                    stage.append((gp, dA, u2))
                if gp >= 3:
                    gq, dAq, u2q = stage.pop(0)
                    h2 = rg.tile([128, 2 * CT], bf16, name="h2",
                                 tag="h2", bufs=2)
                    for i in range(2):
                        nc.vector.tensor_tensor_scan(
                            h2[:, i * CT:(i + 1) * CT],
                            dAq[:, i * CT:(i + 1) * CT],
                            u2q[:, i * CT:(i + 1) * CT],
                            0.0, OP.mult, OP.add)
                    yh2 = rg.tile([128, 2 * CT], bf16, name="yh2",
                                  tag="yh2", bufs=2)
                    eng = nc.gpsimd if gq % 2 == 1 else nc.vector
                    eng.tensor_mul(
                        yh2[:].rearrange("p (i c) -> p i c", i=2),
                        h2[:].rearrange("p (i c) -> p i c", i=2),
                        Crep_b)
                    for i in range(2):
                        g = gq * 2 + i
                        mm(pY[:], BR(f"sum{g}"),
                           yh2[:, i * CT:(i + 1) * CT],
                           g == 0, g == 15)
            yg = rg.tile([128, CT], bf16, name=f"yg{cc}", tag=f"yg{cc}",
                         bufs=1)
            if flags["D_ones"]:
                nc.vector.tensor_add(yg[:], pY[:],
                                     xcb[cc][:, c0:c0 + CT])
            else:
                nc.vector.scalar_tensor_tensor(
                    yg[:], xcb[cc][:, c0:c0 + CT],
                    VP(f"Dp{mi}{cc}"), pY[:], OP.mult, OP.add)
            gz = rg.tile([128, CT], bf16, name=f"gz{cc}", tag=f"gz{cc}",
                         bufs=1)
            nc.gpsimd.tensor_mul(gz[:], yg[:], zsb[cc][:, c0:c0 + CT])
            gzs.append(gz)
        po = psC("po")
        for cc in range(2):
            mm(po[:], BR(f"wout{mi}{cc}"), gzs[cc][:], cc == 0, cc == 1)
        res = s_tile[:, s_c0:s_c0 + CT]
        if is_t:
            nc.vector.tensor_add(res, po[:], xT[:, c0:c0 + CT])
            s2 = rg.tile([128, CT], bf16, name="s2", tag="s2", bufs=1)
            nc.gpsimd.tensor_mul(s2[:], res, res)
            ln_stats(res, s2[:], (bname, "a"), c0)
            ln_finish((bname, "a"), c0)
        else:
            # n/l: write pre-mix value (ym) into the s tile; mix rewrites
            nc.scalar.activation(res, po[:], AF.Copy)

    # ========= factory + scan generator =====================
    # units: fac(0), fac(1), [scan(0), fac(2)], [scan(1), fac(3)], ...
    def g_fs(bname):
        _, mi, _, _, _ = HB[bname]
        is_t = bname[0] == "t"
        s_tile = s_full.get(bname)
        dts_ring = {}
        emit_factory(bname, mi, 0, dts_ring)
        yield
        emit_factory(bname, mi, 1, dts_ring)
        yield
        for ci in range(NT):
            if s_tile is not None:
                st, sc0 = s_tile, ci * CT
            else:
                st = rg.tile([128, CT], bf16, name="sT", tag="sT", bufs=8)
                tiles_s[(bname, ci)] = st
                sc0 = 0
            emit_scan(bname, mi, ci, dts_ring, is_t, st, sc0)
            if ci + 2 < NT:
                emit_factory(bname, mi, ci + 2, dts_ring)
            yield

    tiles_s = {}

    # ================= mix generator (8 units) =======================
    def g_mix(pair):
        nb, lb = pair
        q_n = HB[nb][3]
        q_l = HB[lb][3]
        for c0 in range(0, CBLK, CT):
            cat = {"n": s_full[nb][:, c0:c0 + CT],
                   "l": s_full[lb][:, c0:c0 + CT]}
            # both mix matmuls first: they read cat slices that the res
            # writes below overwrite in place (s tile doubles as ym)
            mss = []
            for mc in range(2):
                pmx = psC("pmx")
                for kc, kk in enumerate(("n", "l")):
                    mm(pmx[:], BR(f"mix{kc}{mc}"), cat[kk], kc == 0,
                       kc == 1)
                ms = rg.tile([128, CT], bf16, name="ms", tag=f"ms{mc}",
                             bufs=1)
                if flags["mixb_zero"]:
                    nc.scalar.activation(ms[:], pmx[:], AF.Silu)
                else:
                    nc.scalar.activation(ms[:], pmx[:], AF.Silu,
                                         bias=VP(f"mixb{mc}"))
                mss.append(ms)
            # t2 adds also read cat before any res write
            t2s = []
            for mc, key in enumerate(("n", "l")):
                t2 = rg.tile([128, CT], bf16, name="t2", tag=f"t2{mc}",
                             bufs=1)
                nc.vector.tensor_add(t2[:], cat[key], mss[mc][:])
                t2s.append(t2)
            for mc, (key, bn, xk, qo) in enumerate(
                    (("n", nb, "x_n", q_n), ("l", lb, "x_l", q_l))):
                q0 = qo + c0 // W
                raw = rg.tile([64, 8 * 128], f32, name="rawm", tag="raw",
                              bufs=1)
                nc.sync.dma_start(raw[:], aps[xk][:, q0:q0 + 8, :])
                ptx = psC("ptx")
                for i in range(8):
                    nc.tensor.transpose(ptx[:, i * 64:(i + 1) * 64],
                                        raw[:, i * 128:(i + 1) * 128],
                                        I64[:64, :64])
                res = s_full[bn][:, c0:c0 + CT]
                nc.vector.tensor_add(res, t2s[mc][:], ptx[:])
                s2m = rg.tile([128, CT], bf16, name="s2m", tag="s2m",
                              bufs=2)
                nc.gpsimd.tensor_mul(s2m[:], res, res)
                ln_stats(res, s2m[:], (bn, "a"), c0)
            yield

    # ================= phase C generator (16 units) ==================
    def g_C(bname, fin_a):
        _, mi, _, _, j0 = HB[bname]
        is_t = bname[0] == "t"

        def CF(ci):
            c0 = ci * CT
            if is_t:
                src = tiles_s.pop((bname, ci))[:]
            else:
                src = s_full[bname][:, c0:c0 + CT]
            n1 = rg.tile([128, CT], bf16, name="n1", tag="n1", bufs=2)
            ga, gb = (None, None) if flags["an_id"] else \
                (f"ang{mi}", f"anb{mi}")
            ln_apply(src, (bname, "a"), c0, ga, gb, n1[:])
            hh = rg.tile([128, 4 * CT], bf16, name="hh", tag="hh", bufs=1)
            for hp in range(2):
                pf = psA("pf")
                for ci2 in range(2):
                    c4 = hp * 2 + ci2
                    mm(pf[:, ci2 * CT:(ci2 + 1) * CT],
                       BR(f"ff1{mi}")[:, c4 * 128:(c4 + 1) * 128],
                       n1[:], True, True)
                if flags["f1b_zero"]:
                    nc.scalar.activation(
                        hh[:, hp * 2 * CT:(hp + 1) * 2 * CT], pf[:],
                        AF.Prelu, alpha=0.01)
                else:
                    for ci2 in range(2):
                        c4 = hp * 2 + ci2
                        nc.scalar.activation(
                            hh[:, c4 * CT:(c4 + 1) * CT],
                            pf[:, ci2 * CT:(ci2 + 1) * CT], AF.Prelu,
                            bias=VP(f"f1b{mi}{c4}"), alpha=0.01)
            pf2 = psC("pf2")
            for c4 in range(4):
                mm(pf2[:], BR(f"ff2{mi}{c4}"),
                   hh[:, c4 * CT:(c4 + 1) * CT], c4 == 0, c4 == 3)
            sf = rg.tile([128, CT], bf16, name="sf", tag="sf", bufs=2)
            if flags["f2b_zero"]:
                nc.vector.tensor_add(sf[:], pf2[:], n1[:])
            else:
                nc.vector.scalar_tensor_tensor(sf[:], pf2[:],
                                               VP(f"f2b{mi}"),
                                               n1[:], OP.add, OP.add)
            s2f = rg.tile([128, CT], bf16, name="s2f", tag="s2f", bufs=1)
            nc.gpsimd.tensor_mul(s2f[:], sf[:], sf[:])
            ln_stats(sf[:], s2f[:], (bname, "f"), c0)
            return sf

        def CT_out(ci, sf):
            c0 = ci * CT
            ga, gb = (None, None) if flags["fln_id"] else \
                (f"flg{mi}", f"flb{mi}")
            n2 = rg.tile([128, CT], f32, name="n2", tag="n2", bufs=1)
            ln_apply(sf[:], (bname, "f"), c0, ga, gb, n2[:])
            pto = psC("pto")
            for c in range(4):
                nc.tensor.transpose(pto[:, c * 128:(c + 1) * 128],
                                    n2[:, c * 128:(c + 1) * 128],
                                    I128)
            ot = rg.tile([128, CT], f32, name="ot", tag="ot", bufs=1)
            nc.vector.tensor_copy(ot[:], pto[:])
            q0 = c0 // W
            for qh in range(2):
                nc.sync.dma_start(
                    aps["out"][:, j0 + q0:j0 + q0 + 8, :]
                    .rearrange("t (c q) d -> q t c d", c=4)[qh],
                    ot[qh * 64:(qh + 1) * 64, :]
                    .rearrange("t (c d) -> t c d", c=4))

        # software-pipelined: finishes run one tile ahead of the applies
        # that consume their DRAM-bounced scale rows, so the rnm2 load DMA
        # never holds the DMA queue waiting on a just-issued write.
        if fin_a:
            ln_finish((bname, "a"), 0)
        sf_prev = None
        for ci in range(NT):
            if is_t:
                while (bname, ci) not in tiles_s:
                    yield   # scan hasn't produced this tile yet; spin
            if fin_a and ci + 1 < NT:
                ln_finish((bname, "a"), (ci + 1) * CT)
            sf = CF(ci)
            if ci >= 1:
                ln_finish((bname, "f"), (ci - 1) * CT)
            yield
            if sf_prev is not None:
                CT_out(ci - 1, sf_prev)
                yield
            sf_prev = sf
        ln_finish((bname, "f"), (NT - 1) * CT)
        yield
        CT_out(NT - 1, sf_prev)
        yield

    # ===================== master schedule ===========================
    with tc.tile_pool(name="a_sq", bufs=1) as a_sqp:
        cq = []   # persistent queue of pending C generators

        def A1(bn):
            return g_a1(bn, a_sqp)

        # order: n0, l0, t0, n1, l1, t1, t2, t3 — a family's C phase fully
        # drains inside t-block windows before its s/sT tag is reused.
        _drain(A1("n0"))
        _drain(g_fs("n0"))
        _drain(A1("l0"))
        _drain(g_fs("l0"))
        _weave([(A1("t0"), 1), (g_mix(("n0", "l0")), 1)])
        C_n0 = GStream(g_C("n0", True))
        C_l0 = GStream(g_C("l0", True))
        C_t0 = GStream(_delay(5, g_C("t0", False)))
        cq.extend([C_n0, C_l0, C_t0])
        _weave_until(g_fs("t0"), cq, w_master=1, w_c=3)
        C_n0.finish()          # s_n tag is rewritten by scan(n1)
        _drain(A1("n1"))       # silu window: no C (act-table isolation)
        _weave_until(g_fs("n1"), cq, w_master=1, w_c=3)
        C_l0.finish()          # s_l tag is rewritten by scan(l1)
        _drain(A1("l1"))
        _weave_until(g_fs("l1"), cq, w_master=1, w_c=3)
        _weave([(A1("t1"), 1), (g_mix(("n1", "l1")), 1)])
        C_prev = C_t0
        cq.extend([GStream(g_C("n1", True)), GStream(g_C("l1", True))])
        for bn, nxt in (("t1", "t2"), ("t2", "t3"), ("t3", None)):
            C_prev.finish()    # sT ring slots reused by scan(bn)
            C_cur = GStream(_delay(5, g_C(bn, False)))
            cq.append(C_cur)
            _weave_until(g_fs(bn), cq, w_master=1, w_c=4)
            if nxt is not None:
                _drain(A1(nxt))
            C_prev = C_cur
        for g in cq:
            g.finish()


def _build_program(wp, vp, bpk, flags):
    nc = bacc.Bacc("TRN2", target_bir_lowering=False, debug=False,
                   num_devices=N_CORES)
    aps = {}
    aps["x_n"] = nc.dram_tensor("x_n", [W, N, D], f32,
                                kind="ExternalInput").ap()
    aps["x_t"] = nc.dram_tensor("x_t", [W, E, D], f32,
                                kind="ExternalInput").ap()
    aps["x_l"] = nc.dram_tensor("x_l", [W, N, D], f32,
                                kind="ExternalInput").ap()
    aps["wpack"] = nc.dram_tensor("wpack", [128, wp.n], f32,
                                  kind="ExternalInput").ap()
    aps["vpack"] = nc.dram_tensor("vpack", [128, vp.n], f32,
                                  kind="ExternalInput").ap()
    aps["bpack"] = nc.dram_tensor("bpack", [128, bpk.n], bf16,
                                  kind="ExternalInput").ap()
    aps["out"] = nc.dram_tensor("out", [W, 2 * N + E, D], f32,
                                kind="ExternalOutput").ap()
    for bname, _, _, _, _ in HBLOCKS:
        aps[f"scr_bc_{bname}"] = nc.dram_tensor(
            f"scr_bc_{bname}", [40, CBLK], bf16).ap()
        aps[f"scr_dtx_{bname}"] = nc.dram_tensor(
            f"scr_dtx_{bname}", [2, NT, 8, 16 * CT], bf16).ap()
        for ph in ("a", "f"):
            aps[f"scr_rn_{bname}_{ph}"] = nc.dram_tensor(
                f"scr_rn_{bname}_{ph}", [2, 128, NW], bf16).ap()

    with tile.TileContext(nc) as tc:
        with ExitStack() as ctx:
            _emit(ctx, tc, nc, aps, wp, vp, bpk, flags)
    nc.compile()
    return nc


_CACHE = {}


def kernel(**inputs):
    wp, vp, bpk, flags = _host_pack(inputs)
    if "prog" not in _CACHE:
        _CACHE["prog"] = _build_program(wp, vp, bpk, flags)
    nc = _CACHE["prog"]
    wpack, vpack = wp.build(), vp.build()
    bpack = bpk.build().astype(ml_dtypes.bfloat16)
    in_maps = []
    for b in range(B):
        in_maps.append({
            "x_n": np.ascontiguousarray(inputs["x_node"][b]),
            "x_t": np.ascontiguousarray(inputs["x_trace"][b]),
            "x_l": np.ascontiguousarray(inputs["x_log"][b]),
            "wpack": wpack,
            "vpack": vpack,
            "bpack": bpack,
        })
    res = run_bass_kernel_spmd(nc, in_maps, list(range(N_CORES)))
    out = np.stack([res.results[b]["out"] for b in range(B)], axis=0)
    return out.astype(np.float32)


# revision 27
# speedup vs baseline: 1.2319x; 1.0858x over previous
"""Trainium2 Bass kernel for nn_Encoder (tri-modal Mamba encoder), v3.

kernel(**inputs) takes FULL unsharded numpy inputs and returns the FULL
output (B, W, 2N+E, D). Batch B=8 is sharded across 8 NeuronCores (pure
data parallel, no collectives); params are replicated.

v3 vs v2 (same math, new orchestration):
- Half-block (64-seq) processing units, software-pipelined end to end:
  the AddNorm+FFN+output phase (C) of each half-block is emitted as a
  persistent generator that drains into whatever later window has engine
  slack, so the serial phase-C tail is gone.
- LayerNorm stats finish per column-tile with rsqrt = exp(-0.5*ln(v+eps))
  so interleaved phases stay inside the natural_log_exp activation-table
  set (Silu windows are kept separate: A1 and mix).
- s / LN stats / LN scale factors live in SBUF; ym merged into s tiles.
- One shared 8-bank PSUM tag discipline (psA 2x[128,1024]f32,
  psY 2x[128,512], psC 2x[128,512]) across all phases.
- Output path: 4 batched f32 transposes per 512-col tile, one copy, one
  DMA (q t d scatter) straight from the LN-f apply.
"""

import functools

import ml_dtypes
import numpy as np
from contextlib import ExitStack

import concourse.bass as bass
import concourse.tile as tile
import concourse.bacc as bacc_mod
import concourse.hw_specs as hw_specs_mod
from concourse import bacc, mybir
from concourse.bass_utils import run_bass_kernel_spmd

# Prefer the exp+ln combined activation-table set so the Exp/Ln mix
# (softplus factory, scan decay, LN rsqrt) resolves to ONE table and the
# act-table load pass stops thrashing between exp_and_others/natural_log.
_GAT_ORIG = hw_specs_mod.get_activation_tables.__wrapped__


@functools.cache
def _gat_reordered(arch):
    t = dict(_GAT_ORIG(arch))
    pref = [k for k in ("natural_log_exp_and_others",) if k in t]
    return {k: t[k] for k in pref + [k for k in t if k not in pref]}


hw_specs_mod.get_activation_tables = _gat_reordered
bacc_mod.get_activation_tables = _gat_reordered

D, DI, SS, KK, RR = 128, 256, 16, 4, 8
B, W, N, E = 8, 64, 128, 256
Q = 64                       # seqs per half-block
CBLK = Q * W                 # 4096 cols per half-block
CT = 512                     # column tile (8 seqs)
NT = CBLK // CT              # 8 tiles per half-block
NW = CBLK // 128             # 32 stat cols per partition
f32 = mybir.dt.float32
f32r = mybir.dt.float32r
bf16 = mybir.dt.bfloat16
AF = mybir.ActivationFunctionType
OP = mybir.AluOpType

# (name, modality, input key, q offset, output entity offset)
HBLOCKS = [("n0", 0, "x_n", 0, 0), ("n1", 0, "x_n", 64, 64),
           ("l0", 2, "x_l", 0, 384), ("l1", 2, "x_l", 64, 448),
           ("t0", 1, "x_t", 0, 128), ("t1", 1, "x_t", 64, 192),
           ("t2", 1, "x_t", 128, 256), ("t3", 1, "x_t", 192, 320)]
HB = {b[0]: b for b in HBLOCKS}
N_CORES = 8
LN_EPS = 1e-5


class Pack:
    def __init__(self):
        self.cols = []
        self.off = {}
        self.n = 0

    def add(self, name, arr, dtype=np.float32):
        arr = np.asarray(arr, dtype)
        assert arr.ndim == 2 and arr.shape[0] <= 128
        a = np.zeros((128, arr.shape[1]), dtype)
        a[: arr.shape[0]] = arr
        self.off[name] = (self.n, arr.shape[1])
        self.cols.append(a)
        self.n += arr.shape[1]

    def build(self):
        return np.concatenate(self.cols, axis=1)


def _host_pack(inp):
    """Returns (wp f32-staged-to-f32r, vp f32, bp bf16, flags)."""
    flags = {}
    dtb = np.asarray(inp["mp_dt_b"], np.float64)
    flags["dtb_const"] = float(dtb.flat[0]) if np.ptp(dtb) < 1e-12 else None
    flags["D_ones"] = bool(np.allclose(np.asarray(inp["mp_D"]), 1.0))
    flags["convb_zero"] = bool(np.all(np.asarray(inp["mp_conv_b"]) == 0.0))
    flags["f1b_zero"] = bool(np.all(np.asarray(inp["ff1_b"]) == 0.0))
    flags["f2b_zero"] = bool(np.all(np.asarray(inp["ff2_b"]) == 0.0))
    flags["an_id"] = bool(np.all(np.asarray(inp["an_g"]) == 1.0)
                          and np.all(np.asarray(inp["an_b"]) == 0.0))
    flags["fln_id"] = bool(np.all(np.asarray(inp["fln_g"]) == 1.0)
                           and np.all(np.asarray(inp["fln_b"]) == 0.0))
    flags["mixb_zero"] = bool(np.all(np.asarray(inp["mix_b"]) == 0.0))
    A = -np.exp(np.asarray(inp["mp_Alog"], np.float64))      # (3, DI, S)
    flags["A_shared"] = bool(
        np.ptp(A, axis=(0, 1)).max() < 1e-9 * np.abs(A).max())

    bp = Pack()   # bf16 weights
    for g in range(16):
        sm = np.zeros((128, 128), np.float32)
        for k in range(128):
            sm[k, g * 8 + k // 16] = 1.0
        bp.add(f"sum{g}", sm)

    def delta_A(Am, cc, g):
        dl = np.zeros((128, 128), np.float32)
        for j in range(128):
            dl[g * 8 + j // 16, j] = Am[cc * 128 + g * 8 + j // 16, j % 16]
        return dl

    if flags["A_shared"]:
        for g in range(16):
            bp.add(f"dA{g}", delta_A(A[0], 0, g))
    else:
        for m in range(3):
            for cc in range(2):
                for g in range(16):
                    bp.add(f"dA{m}{cc}{g}", delta_A(A[m], cc, g))

    wp = Pack()   # fp32 staged -> f32r on device
    vp = Pack()   # fp32 per-partition vectors
    for m in range(3):
        bp.add(f"win{m}", inp["mp_in"][m])                   # (D, 512)
        wxp = inp["mp_xproj"][m]                             # (DI, 40)
        for cc in range(2):
            bp.add(f"bc{m}{cc}", wxp[cc * 128:(cc + 1) * 128])
        dtw = inp["mp_dt_w"][m]                              # (R, DI)
        for cc in range(2):
            bp.add(f"dtw{m}{cc}", dtw[:, cc * 128:(cc + 1) * 128])
        wout = inp["mp_out"][m]                              # (DI, D)
        for cc in range(2):
            bp.add(f"wout{m}{cc}", wout[cc * 128:(cc + 1) * 128])
        bp.add(f"ff1{m}", inp["ff1_w"][m])                   # (D, 512)
        ff2 = inp["ff2_w"][m]                                # (4D, D)
        for c4 in range(4):
            bp.add(f"ff2{m}{c4}", ff2[c4 * 128:(c4 + 1) * 128])
    mixw = inp["mix_w"]
    for kc in range(2):
        for mc in range(2):
            bp.add(f"mix{kc}{mc}", mixw[kc * 128:(kc + 1) * 128,
                                        mc * 128:(mc + 1) * 128])
    wp.add("onesD", np.full((128, 1), 1.0 / D, np.float32))
    bp.add("onesDb", np.full((128, 1), 1.0 / D, np.float32))
    wp.add("ones1", np.ones((1, 128), np.float32))

    vp.add("eps", np.full((128, 1), LN_EPS, np.float32))
    if flags["dtb_const"] is not None:
        vp.add("dtbc", np.full((128, 1), flags["dtb_const"], np.float32))
    vp.add("I64", np.eye(64, dtype=np.float32))
    vp.add("I128", np.eye(128, dtype=np.float32))
    for m in range(3):
        cw = inp["mp_conv_w"][m]
        for cc in range(2):
            sl = slice(cc * 128, (cc + 1) * 128)
            vp.add(f"cw{m}{cc}", cw[sl])                     # 4 cols
            if not flags["convb_zero"]:
                vp.add(f"cb{m}{cc}", inp["mp_conv_b"][m][sl, None])
            if flags["dtb_const"] is None:
                vp.add(f"dtb{m}{cc}", inp["mp_dt_b"][m][sl, None])
            if not flags["D_ones"]:
                vp.add(f"Dp{m}{cc}", inp["mp_D"][m][sl, None])
        if not flags["f1b_zero"]:
            for c4 in range(4):
                vp.add(f"f1b{m}{c4}",
                       inp["ff1_b"][m][c4 * 128:(c4 + 1) * 128, None])
        if not flags["f2b_zero"]:
            vp.add(f"f2b{m}", inp["ff2_b"][m][:, None])
        if not flags["an_id"]:
            vp.add(f"ang{m}", inp["an_g"][m][:, None])
            vp.add(f"anb{m}", inp["an_b"][m][:, None])
        if not flags["fln_id"]:
            vp.add(f"flg{m}", inp["fln_g"][m][:, None])
            vp.add(f"flb{m}", inp["fln_b"][m][:, None])
    if not flags["mixb_zero"]:
        for mc in range(2):
            vp.add(f"mixb{mc}", inp["mix_b"][mc * 128:(mc + 1) * 128, None])
    return wp, vp, bp, flags


def _drain(g):
    for _ in g:
        pass


def _weave(streams):
    """streams: (gen, weight) pairs; round-robin to exhaustion."""
    live = [[iter(g), w] for g, w in streams]
    while live:
        for ent in list(live):
            g, w = ent
            for _ in range(w):
                try:
                    next(g)
                except StopIteration:
                    live.remove(ent)
                    break


class GStream:
    def __init__(self, g):
        self.g = iter(g)
        self.done = False

    def step(self):
        if self.done:
            return False
        try:
            next(self.g)
            return True
        except StopIteration:
            self.done = True
            return False

    def finish(self):
        while self.step():
            pass


def _weave_until(master, cs, w_master=1, w_c=1):
    """Interleave master with the persistent queue `cs` (GStream list).
    Returns when master is exhausted; cs keeps its remaining state."""
    m = iter(master)
    while True:
        for _ in range(w_master):
            try:
                next(m)
            except StopIteration:
                return
        budget = w_c
        while budget > 0 and cs:
            if cs[0].step():
                budget -= 1
            else:
                cs.pop(0)


def _delay(n, g):
    for _ in range(n):
        yield
    yield from g


def _emit(ctx, tc, nc, aps, wp, vp, bpk, flags):
    wpool = ctx.enter_context(tc.tile_pool(name="weights", bufs=1))
    wr = wpool.tile([128, wp.n], f32r, name="wr", tag="wr")
    vec = wpool.tile([128, vp.n], f32, name="vec", tag="vec")
    nc.sync.dma_start(vec[:], aps["vpack"][:])
    bw = wpool.tile([128, bpk.n], bf16, name="bw", tag="bw")
    nc.sync.dma_start(bw[:], aps["bpack"][:])
    with tc.tile_pool(name="wstage", bufs=1) as stpool:
        wstage = stpool.tile([128, wp.n], f32, name="wstage")
        nc.sync.dma_start(wstage[:], aps["wpack"][:])
        for o in range(0, wp.n, 8192):
            e = min(wp.n, o + 8192)
            nc.vector.tensor_copy(wr[:, o:e], wstage[:, o:e])

    def WR(name):
        o, c = wp.off[name]
        return wr[:, o:o + c]

    def VP(name):
        o, c = vp.off[name]
        return vec[:, o:o + c]

    def BR(name):
        o, c = bpk.off[name]
        return bw[:, o:o + c]

    def mm(psum_ap, lhsT_ap, rhs_ap, start, stop, kp=128):
        nc.tensor.matmul(psum_ap, lhsT_ap[:kp, :], rhs_ap[:kp, :],
                         start=start, stop=stop)

    def dAW(mi, cc, g):
        return BR(f"dA{g}" if flags["A_shared"] else f"dA{mi}{cc}{g}")

    I64 = VP("I64")
    I128 = VP("I128")

    # ---- long-lived SBUF state --------------------------------------
    blk = ctx.enter_context(tc.tile_pool(name="blk", bufs=1))
    # s tiles: n/l halves need full (128, CBLK); mix writes them in place
    # over the ym values (same storage). t halves use a small ring.
    s_full = {nm: blk.tile([128, CBLK], bf16, name=f"s_{nm}",
                           tag=f"s_{nm[0]}")
              for nm in ("n0", "n1", "l0", "l1")}  # tag per family: 2 tags
    statp = ctx.enter_context(tc.tile_pool(name="stat", bufs=1))
    stat, rnm = {}, {}
    for bname, _, _, _, _ in HBLOCKS:
        for ph in ("a", "f"):
            stat[(bname, ph)] = statp.tile(
                [128, 2 * NW], f32, name=f"st_{bname}{ph}",
                tag=f"st_{bname}{ph}")
            rnm[(bname, ph)] = statp.tile(
                [128, 2 * NW], bf16, name=f"rn_{bname}{ph}",
                tag=f"rn_{bname}{ph}")

    # ---- shared PSUM tags (8 banks total) ---------------------------
    ps = ctx.enter_context(tc.tile_pool(name="ps", bufs=1, space="PSUM"))

    def psA(name):
        # scan/A1/factory-side wide psum (2 banks)
        return ps.tile([128, 2 * CT], f32, name=name, tag="psA", bufs=1)

    def psAC(name):
        # C-side wide psum (2 banks) - decoupled from the scan pipeline
        return ps.tile([128, 2 * CT], f32, name=name, tag="psAC", bufs=1)

    def psS(name, side):
        return ps.tile([1, 2 * CT], f32, name=name,
                       tag="psA" if side == "s" else "psAC", bufs=1)

    def psY(name):
        return ps.tile([128, CT], f32, name=name, tag="psY", bufs=2)

    def psC(name):
        # scan/A1/mix-side narrow psum (1 bank)
        return ps.tile([128, CT], f32, name=name, tag="psC", bufs=1)

    def psCC(name):
        # C-side narrow psum (1 bank)
        return ps.tile([128, CT], f32, name=name, tag="psCC", bufs=1)

    rg = ctx.enter_context(tc.tile_pool(name="rg", bufs=1))

    # ---- LN helpers -------------------------------------------------
    def ln_stats(src_ap, sq_ap, statT, c0, side="s"):
        ob = BR("onesDb")[:, 0:1]
        of = WR("onesD")[:, 0:1]
        pmq = psS("pmq", side)
        mm(pmq[:, 0:CT], ob if src_ap.dtype == bf16 else of, src_ap,
           True, True)
        mm(pmq[:, CT:2 * CT], ob if sq_ap.dtype == bf16 else of, sq_ap,
           True, True)
        sst = rg.tile([1, 2 * CT], f32, name="sst", tag="sst", bufs=1)
        nc.scalar.activation(sst[:], pmq[:], AF.Copy)
        p0 = (c0 // CT) * 16
        nc.sync.dma_start(
            stat[statT][p0:p0 + 16, :].rearrange("p (h w) -> h p w", h=2),
            sst[:].rearrange("x (h p w) -> x h p w", h=2, p=16))

    def ln_finish(statT, c0):
        """Finish LN scale factors for one column tile (16 partitions)."""
        p0 = (c0 // CT) * 16
        sT = stat[statT]
        rT = rnm[statT]
        m_t = sT[p0:p0 + 16, 0:NW]
        q_t = sT[p0:p0 + 16, NW:2 * NW]
        var = rg.tile([16, NW], f32, name="var", tag="lnvar", bufs=2)
        nc.vector.tensor_mul(var[:], m_t, m_t)
        nc.vector.tensor_sub(var[:], q_t, var[:])
        lnv = rg.tile([16, NW], f32, name="lnv", tag="lnlnv", bufs=2)
        nc.scalar.activation(lnv[:], var[:], AF.Ln, bias=VP("eps")[0:16])
        r_t = rg.tile([16, NW], f32, name="lnr", tag="lnr", bufs=2)
        nc.scalar.activation(r_t[:], lnv[:], AF.Exp, scale=-0.5)
        nmr = rg.tile([16, NW], f32, name="nmr", tag="lnnmr", bufs=2)
        nc.vector.tensor_mul(nmr[:], m_t, r_t[:])
        nc.vector.tensor_scalar(nmr[:], nmr[:], -1.0, None, OP.mult)
        nc.vector.tensor_copy(rT[p0:p0 + 16, 0:NW], r_t[:])
        nc.vector.tensor_copy(rT[p0:p0 + 16, NW:2 * NW], nmr[:])
        bn, ph = statT
        nc.sync.dma_start(
            aps[f"scr_rn_{bn}_{ph}"][:, p0:p0 + 16, :],
            rT[p0:p0 + 16, :].rearrange("p (h w) -> h p w", h=2))

    def ln_apply(src_ap, rnmT, c0, gk, bk, out_ap):
        p0 = (c0 // CT) * 16
        rnm2 = rg.tile([128, 2 * CT], bf16, name="rnm2", tag="rnm2",
                       bufs=2)
        bn, ph = rnmT
        nc.gpsimd.dma_start(
            rnm2[:].rearrange("j (h ab) -> j h ab", h=2),
            aps[f"scr_rn_{bn}_{ph}"][:, p0:p0 + 16, :]
            .rearrange("h p w -> h (p w)")
            .unsqueeze(0).broadcast_to((128, 2, CT)))
        t1 = rg.tile([128, CT], bf16, name="t1", tag="t1", bufs=2)
        nc.vector.tensor_mul(t1[:], src_ap, rnm2[:, 0:CT])
        if gk is None:
            nc.vector.tensor_add(out_ap, t1[:], rnm2[:, CT:2 * CT])
        else:
            nc.vector.tensor_add(t1[:], t1[:], rnm2[:, CT:2 * CT])
            nc.vector.tensor_scalar(out_ap, t1[:], VP(gk), VP(bk),
                                    OP.mult, OP.add)

    tiles = {}

    # ================= phase A1 (8 units) ============================
    def g_a1(bname, sqp):
        _, mi, xkey, q_off, _ = HB[bname]
        xcb = [sqp.tile([128, CBLK], bf16, name=f"xcb{cc}", tag=f"xcb{cc}")
               for cc in range(2)]
        zsb = [sqp.tile([128, CBLK], bf16, name=f"zsb{cc}", tag=f"zsb{cc}")
               for cc in range(2)]
        xT = sqp.tile([128, CBLK], bf16, name="xT", tag="xT")
        tiles[bname] = (xcb, zsb, xT)
        for c0 in range(0, CBLK, CT):
            q0 = c0 // W
            raw = rg.tile([64, 8 * 128], f32, name="raw", tag="raw", bufs=1)
            nc.sync.dma_start(raw[:],
                              aps[xkey][:, q_off + q0:q_off + q0 + 8, :])
            pt = psC("pt")
            for i in range(8):
                nc.tensor.transpose(pt[:, i * 64:(i + 1) * 64],
                                    raw[:, i * 128:(i + 1) * 128],
                                    I64[:64, :64])
            nc.scalar.activation(xT[:, c0:c0 + CT], pt[:], AF.Copy)
            xt_t = xT[:, c0:c0 + CT]
            pxc2 = psA("pxc2")
            for cc in range(2):
                mm(pxc2[:, cc * CT:(cc + 1) * CT],
                   BR(f"win{mi}")[:, cc * 128:(cc + 1) * 128], xt_t,
                   True, True)
            pz2 = psA("pz2")
            for cc in range(2):
                mm(pz2[:, cc * CT:(cc + 1) * CT],
                   BR(f"win{mi}")[:, (2 + cc) * 128:(3 + cc) * 128],
                   xt_t, True, True)
            for cc in range(2):
                nc.scalar.activation(zsb[cc][:, c0:c0 + CT],
                                     pz2[:, cc * CT:(cc + 1) * CT],
                                     AF.Silu)
            zc = rg.tile([128, 2 * CT], bf16, name="zc", tag="zc", bufs=1)
            nc.scalar.activation(zc[:], pxc2[:], AF.Copy)
            acc2 = rg.tile([128, 2 * CT], f32, name="acc2", tag="acc2",
                           bufs=1)
            for cc in range(2):
                pzv = zc[:, cc * CT:(cc + 1) * CT]
                accv = acc2[:, cc * CT:(cc + 1) * CT]
                cw = VP(f"cw{mi}{cc}")
                srcr = pzv.rearrange("p (q t) -> p q t", t=W)
                accr = accv.rearrange("p (q t) -> p q t", t=W)
                nc.vector.tensor_scalar(accv, pzv, cw[:, 3:4], None,
                                        OP.mult)
                for k in range(3):
                    sh = 3 - k
                    nc.vector.scalar_tensor_tensor(
                        accr[:, :, sh:W], srcr[:, :, 0:W - sh],
                        cw[:, k:k + 1], accr[:, :, sh:W],
                        OP.mult, OP.add)
            for cc in range(2):
                bias = (None if flags["convb_zero"]
                        else VP(f"cb{mi}{cc}"))
                if bias is None:
                    nc.scalar.activation(xcb[cc][:, c0:c0 + CT],
                                         acc2[:, cc * CT:(cc + 1) * CT],
                                         AF.Silu)
                else:
                    nc.scalar.activation(xcb[cc][:, c0:c0 + CT],
                                         acc2[:, cc * CT:(cc + 1) * CT],
                                         AF.Silu, bias=bias)
            pbc = psC("pbc")
            for cc in range(2):
                mm(pbc[:40, :], BR(f"bc{mi}{cc}"),
                   xcb[cc][:, c0:c0 + CT], cc == 0, cc == 1)
            bcs = rg.tile([40, CT], bf16, name="bcs", tag="bcs", bufs=2)
            nc.scalar.activation(bcs[:], pbc[:40, :], AF.Copy)
            nc.sync.dma_start(aps[f"scr_bc_{bname}"][:, c0:c0 + CT],
                              bcs[:])
            yield

    # ============ dt factory: one tile -> dts ring slot ==============
    def emit_factory(bname, mi, ci, dts_ring):
        xcb, _, _ = tiles[bname]
        c0 = ci * CT
        dtin = rg.tile([8, CT], bf16, name="dtin", tag="dtin", bufs=4)
        nc.sync.dma_start(dtin[:],
                          aps[f"scr_bc_{bname}"][0:8, c0:c0 + CT])
        pd = psA("pd")
        for cc in range(2):
            mm(pd[:, cc * CT:(cc + 1) * CT], BR(f"dtw{mi}{cc}"),
               dtin[:], True, True, kp=8)
        ez = rg.tile([128, 2 * CT], bf16, name="ez", tag="ez", bufs=1)
        if flags["dtb_const"] is not None:
            nc.scalar.activation(ez[:], pd[:], AF.Exp, bias=VP("dtbc"))
        else:
            for cc in range(2):
                nc.scalar.activation(
                    ez[:, cc * CT:(cc + 1) * CT],
                    pd[:, cc * CT:(cc + 1) * CT],
                    AF.Exp, bias=VP(f"dtb{mi}{cc}"))
        dts = rg.tile([128, 2 * CT], bf16, name="dts", tag="dts", bufs=3)
        nc.scalar.activation(dts[:], ez[:], AF.Ln, bias=1.0)
        dtx = rg.tile([128, 2 * CT], bf16, name="dtx", tag="dtx", bufs=2)
        for cc in range(2):
            nc.vector.tensor_mul(dtx[:, cc * CT:(cc + 1) * CT],
                                 dts[:, cc * CT:(cc + 1) * CT],
                                 xcb[cc][:, c0:c0 + CT])
        for cc in range(2):
            nc.sync.dma_start(
                aps[f"scr_dtx_{bname}"][cc, ci]
                .rearrange("p (g c) -> g p c", g=16),
                dtx[:, cc * CT:(cc + 1) * CT])
        dtv = dts[:].rearrange("p (x t) -> p x t", t=W)
        nc.vector.tensor_scalar(dtv[:, :, 0:1], dtv[:, :, 0:1],
                                0.0, 1.0e4, OP.mult, OP.add)
        dts_ring[ci] = dts

    # ============ scan unit (one tile) ===============================
    def emit_scan(bname, mi, ci, dts_ring, is_t, s_tile, s_c0):
        xcb, zsb, xT = tiles[bname]
        c0 = ci * CT
        dts = dts_ring.pop(ci)
        Brep = rg.tile([128, CT], bf16, name="Brep", tag="Brep", bufs=3)
        nc.sync.dma_start(
            Brep[:],
            aps[f"scr_bc_{bname}"][8:24, c0:c0 + CT]
            .unsqueeze(0).broadcast_to((8, 16, CT)))
        Crep = rg.tile([128, CT], bf16, name="Crep", tag="Crep", bufs=3)
        nc.sync.dma_start(
            Crep[:],
            aps[f"scr_bc_{bname}"][24:40, c0:c0 + CT]
            .unsqueeze(0).broadcast_to((8, 16, CT)))
        ues = []
        for cc in range(2):
            halves = []
            for hf in range(2):
                ueh = rg.tile([128, 8 * CT], bf16, name="ueh",
                              tag=f"ueh{hf}", bufs=2 - hf)
                nc.gpsimd.dma_start(
                    ueh[:],
                    aps[f"scr_dtx_{bname}"]
                    [cc, ci, :, hf * 8 * CT:(hf + 1) * 8 * CT]
                    .unsqueeze(1).broadcast_to((8, 16, 8 * CT)))
                halves.append(ueh)
            ues.append(halves)
        Brep_b = Brep[:].unsqueeze(1).broadcast_to((128, 2, CT))
        Crep_b = Crep[:].unsqueeze(1).broadcast_to((128, 2, CT))
        gzs = []
        for cc in range(2):
            pY = psY(f"pY{cc}")
            stage = []
            for gp in range(11):
                if gp < 8:
                    pP = psA("pP")
                    for i in range(2):
                        g = gp * 2 + i
                        mm(pP[:, i * CT:(i + 1) * CT],
                           dAW(mi, cc, g),
                           dts[:, cc * CT:(cc + 1) * CT],
                           True, True)
                    dA = rg.tile([128, 2 * CT], bf16, name="dA",
                                 tag="dA", bufs=3)
                    nc.scalar.activation(dA[:], pP[:], AF.Exp)
                    u2 = rg.tile([128, 2 * CT], bf16, name="u2",
                                 tag="u2", bufs=3)
                    ueh = ues[cc][gp // 4]
                    sl = (gp % 4) * 2 * CT
                    eng = nc.gpsimd if gp % 4 == 3 else nc.vector
                    eng.tensor_mul(
                        u2[:].rearrange("p (i c) -> p i c", i=2),
                        ueh[:, sl:sl + 2 * CT]
                        .rearrange("p (i c) -> p i c", i=2),
                        Brep_b)
                    stage.append((gp, dA, u2))
                if gp >= 3:
                    gq, dAq, u2q = stage.pop(0)
                    h2 = rg.tile([128, 2 * CT], bf16, name="h2",
                                 tag="h2", bufs=2)
                    for i in range(2):
                        nc.vector.tensor_tensor_scan(
                            h2[:, i * CT:(i + 1) * CT],
                            dAq[:, i * CT:(i + 1) * CT],
                            u2q[:, i * CT:(i + 1) * CT],
                            0.0, OP.mult, OP.add)
                    yh2 = rg.tile([128, 2 * CT], bf16, name="yh2",
                                  tag="yh2", bufs=2)
                    eng = nc.gpsimd if gq % 2 == 1 else nc.vector
                    eng.tensor_mul(
                        yh2[:].rearrange("p (i c) -> p i c", i=2),
                        h2[:].rearrange("p (i c) -> p i c", i=2),
                        Crep_b)
                    for i in range(2):
                        g = gq * 2 + i
                        mm(pY[:], BR(f"sum{g}"),
                           yh2[:, i * CT:(i + 1) * CT],
                           g == 0, g == 15)
            yg = rg.tile([128, CT], bf16, name=f"yg{cc}", tag=f"yg{cc}",
                         bufs=1)
            if flags["D_ones"]:
                nc.vector.tensor_add(yg[:], pY[:],
                                     xcb[cc][:, c0:c0 + CT])
            else:
                nc.vector.scalar_tensor_tensor(
                    yg[:], xcb[cc][:, c0:c0 + CT],
                    VP(f"Dp{mi}{cc}"), pY[:], OP.mult, OP.add)
            gz = rg.tile([128, CT], bf16, name=f"gz{cc}", tag=f"gz{cc}",
                         bufs=1)
            nc.gpsimd.tensor_mul(gz[:], yg[:], zsb[cc][:, c0:c0 + CT])
            gzs.append(gz)
        po = psC("po")
        for cc in range(2):
            mm(po[:], BR(f"wout{mi}{cc}"), gzs[cc][:], cc == 0, cc == 1)
        res = s_tile[:, s_c0:s_c0 + CT]
        if is_t:
            nc.vector.tensor_add(res, po[:], xT[:, c0:c0 + CT])
            s2 = rg.tile([128, CT], bf16, name="s2", tag="s2", bufs=1)
            nc.gpsimd.tensor_mul(s2[:], res, res)
            ln_stats(res, s2[:], (bname, "a"), c0)
            ln_finish((bname, "a"), c0)
        else:
            # n/l: write pre-mix value (ym) into the s tile; mix rewrites
            nc.scalar.activation(res, po[:], AF.Copy)

    # ========= factory + scan generator =====================
    # units: fac(0), fac(1), [scan(0), fac(2)], [scan(1), fac(3)], ...
    def g_fs(bname):
        _, mi, _, _, _ = HB[bname]
        is_t = bname[0] == "t"
        s_tile = s_full.get(bname)
        dts_ring = {}
        emit_factory(bname, mi, 0, dts_ring)
        yield
        emit_factory(bname, mi, 1, dts_ring)
        yield
        for ci in range(NT):
            if s_tile is not None:
                st, sc0 = s_tile, ci * CT
            else:
                st = rg.tile([128, CT], bf16, name="sT", tag="sT", bufs=8)
                tiles_s[(bname, ci)] = st
                sc0 = 0
            emit_scan(bname, mi, ci, dts_ring, is_t, st, sc0)
            if ci + 2 < NT:
                emit_factory(bname, mi, ci + 2, dts_ring)
            yield

    tiles_s = {}

    # ================= mix generator (8 units) =======================
    def g_mix(pair):
        nb, lb = pair
        q_n = HB[nb][3]
        q_l = HB[lb][3]
        for c0 in range(0, CBLK, CT):
            cat = {"n": s_full[nb][:, c0:c0 + CT],
                   "l": s_full[lb][:, c0:c0 + CT]}
            # both mix matmuls first: they read cat slices that the res
            # writes below overwrite in place (s tile doubles as ym)
            mss = []
            for mc in range(2):
                pmx = psC("pmx")
                for kc, kk in enumerate(("n", "l")):
                    mm(pmx[:], BR(f"mix{kc}{mc}"), cat[kk], kc == 0,
                       kc == 1)
                ms = rg.tile([128, CT], bf16, name="ms", tag=f"ms{mc}",
                             bufs=1)
                if flags["mixb_zero"]:
                    nc.scalar.activation(ms[:], pmx[:], AF.Silu)
                else:
                    nc.scalar.activation(ms[:], pmx[:], AF.Silu,
                                         bias=VP(f"mixb{mc}"))
                mss.append(ms)
            # t2 adds also read cat before any res write
            t2s = []
            for mc, key in enumerate(("n", "l")):
                t2 = rg.tile([128, CT], bf16, name="t2", tag=f"t2{mc}",
                             bufs=1)
                nc.vector.tensor_add(t2[:], cat[key], mss[mc][:])
                t2s.append(t2)
            for mc, (key, bn, xk, qo) in enumerate(
                    (("n", nb, "x_n", q_n), ("l", lb, "x_l", q_l))):
                q0 = qo + c0 // W
                raw = rg.tile([64, 8 * 128], f32, name="rawm", tag="raw",
                              bufs=1)
                nc.sync.dma_start(raw[:], aps[xk][:, q0:q0 + 8, :])
                ptx = psC("ptx")
                for i in range(8):
                    nc.tensor.transpose(ptx[:, i * 64:(i + 1) * 64],
                                        raw[:, i * 128:(i + 1) * 128],
                                        I64[:64, :64])
                res = s_full[bn][:, c0:c0 + CT]
                nc.vector.tensor_add(res, t2s[mc][:], ptx[:])
                s2m = rg.tile([128, CT], bf16, name="s2m", tag="s2m",
                              bufs=2)
                nc.gpsimd.tensor_mul(s2m[:], res, res)
                ln_stats(res, s2m[:], (bn, "a"), c0)
            yield

    # ================= phase C generator (16 units) ==================
    def g_C(bname, fin_a):
        _, mi, _, _, j0 = HB[bname]
        is_t = bname[0] == "t"

        def CF(ci):
            c0 = ci * CT
            if is_t:
                src = tiles_s.pop((bname, ci))[:]
            else:
                src = s_full[bname][:, c0:c0 + CT]
            n1 = rg.tile([128, CT], bf16, name="n1", tag="n1", bufs=2)
            ga, gb = (None, None) if flags["an_id"] else \
                (f"ang{mi}", f"anb{mi}")
            ln_apply(src, (bname, "a"), c0, ga, gb, n1[:])
            hh = rg.tile([128, 4 * CT], bf16, name="hh", tag="hh", bufs=1)
            for hp in range(2):
                pf = psAC("pf")
                for ci2 in range(2):
                    c4 = hp * 2 + ci2
                    mm(pf[:, ci2 * CT:(ci2 + 1) * CT],
                       BR(f"ff1{mi}")[:, c4 * 128:(c4 + 1) * 128],
                       n1[:], True, True)
                if flags["f1b_zero"]:
                    nc.scalar.activation(
                        hh[:, hp * 2 * CT:(hp + 1) * 2 * CT], pf[:],
                        AF.Prelu, alpha=0.01)
                else:
                    for ci2 in range(2):
                        c4 = hp * 2 + ci2
                        nc.scalar.activation(
                            hh[:, c4 * CT:(c4 + 1) * CT],
                            pf[:, ci2 * CT:(ci2 + 1) * CT], AF.Prelu,
                            bias=VP(f"f1b{mi}{c4}"), alpha=0.01)
            pf2 = psCC("pf2")
            for c4 in range(4):
                mm(pf2[:], BR(f"ff2{mi}{c4}"),
                   hh[:, c4 * CT:(c4 + 1) * CT], c4 == 0, c4 == 3)
            sf = rg.tile([128, CT], bf16, name="sf", tag="sf", bufs=2)
            if flags["f2b_zero"]:
                nc.vector.tensor_add(sf[:], pf2[:], n1[:])
            else:
                nc.vector.scalar_tensor_tensor(sf[:], pf2[:],
                                               VP(f"f2b{mi}"),
                                               n1[:], OP.add, OP.add)
            s2f = rg.tile([128, CT], bf16, name="s2f", tag="s2f", bufs=1)
            nc.gpsimd.tensor_mul(s2f[:], sf[:], sf[:])
            ln_stats(sf[:], s2f[:], (bname, "f"), c0, side="C")
            return sf

        def CT_out(ci, sf):
            c0 = ci * CT
            ga, gb = (None, None) if flags["fln_id"] else \
                (f"flg{mi}", f"flb{mi}")
            n2 = rg.tile([128, CT], f32, name="n2", tag="n2", bufs=1)
            ln_apply(sf[:], (bname, "f"), c0, ga, gb, n2[:])
            pto = psCC("pto")
            for c in range(4):
                nc.tensor.transpose(pto[:, c * 128:(c + 1) * 128],
                                    n2[:, c * 128:(c + 1) * 128],
                                    I128)
            ot = rg.tile([128, CT], f32, name="ot", tag="ot", bufs=1)
            nc.vector.tensor_copy(ot[:], pto[:])
            q0 = c0 // W
            for qh in range(2):
                nc.sync.dma_start(
                    aps["out"][:, j0 + q0:j0 + q0 + 8, :]
                    .rearrange("t (c q) d -> q t c d", c=4)[qh],
                    ot[qh * 64:(qh + 1) * 64, :]
                    .rearrange("t (c d) -> t c d", c=4))

        # software-pipelined: finishes run one tile ahead of the applies
        # that consume their DRAM-bounced scale rows, so the rnm2 load DMA
        # never holds the DMA queue waiting on a just-issued write.
        if fin_a:
            ln_finish((bname, "a"), 0)
        sf_prev = None
        for ci in range(NT):
            if is_t:
                while (bname, ci) not in tiles_s:
                    yield   # scan hasn't produced this tile yet; spin
            if fin_a and ci + 1 < NT:
                ln_finish((bname, "a"), (ci + 1) * CT)
            sf = CF(ci)
            if ci >= 1:
                ln_finish((bname, "f"), (ci - 1) * CT)
            yield
            if sf_prev is not None:
                CT_out(ci - 1, sf_prev)
                yield
            sf_prev = sf
        ln_finish((bname, "f"), (NT - 1) * CT)
        yield
        CT_out(NT - 1, sf_prev)
        yield

    # ===================== master schedule ===========================
    with tc.tile_pool(name="a_sq", bufs=1) as a_sqp:
        cq = []   # persistent queue of pending C generators

        def A1(bn):
            return g_a1(bn, a_sqp)

        # order: n0, l0, t0, n1, l1, t1, t2, t3 — a family's C phase fully
        # drains inside t-block windows before its s/sT tag is reused.
        _drain(A1("n0"))
        _drain(g_fs("n0"))
        _drain(A1("l0"))
        _drain(g_fs("l0"))
        _weave([(A1("t0"), 1), (g_mix(("n0", "l0")), 1)])
        C_n0 = GStream(g_C("n0", True))
        C_l0 = GStream(g_C("l0", True))
        C_t0 = GStream(_delay(5, g_C("t0", False)))
        cq.extend([C_n0, C_l0, C_t0])
        _weave_until(g_fs("t0"), cq, w_master=1, w_c=3)
        C_n0.finish()          # s_n tag is rewritten by scan(n1)
        _drain(A1("n1"))       # silu window: no C (act-table isolation)
        _weave_until(g_fs("n1"), cq, w_master=1, w_c=3)
        C_l0.finish()          # s_l tag is rewritten by scan(l1)
        _drain(A1("l1"))
        _weave_until(g_fs("l1"), cq, w_master=1, w_c=3)
        _weave([(A1("t1"), 1), (g_mix(("n1", "l1")), 1)])
        C_prev = C_t0
        cq.extend([GStream(g_C("n1", True)), GStream(g_C("l1", True))])
        for bn, nxt in (("t1", "t2"), ("t2", "t3"), ("t3", None)):
            C_prev.finish()    # sT ring slots reused by scan(bn)
            C_cur = GStream(_delay(5, g_C(bn, False)))
            cq.append(C_cur)
            _weave_until(g_fs(bn), cq, w_master=1, w_c=4)
            if nxt is not None:
                _drain(A1(nxt))
            C_prev = C_cur
        for g in cq:
            g.finish()


def _build_program(wp, vp, bpk, flags):
    nc = bacc.Bacc("TRN2", target_bir_lowering=False, debug=False,
                   num_devices=N_CORES)
    aps = {}
    aps["x_n"] = nc.dram_tensor("x_n", [W, N, D], f32,
                                kind="ExternalInput").ap()
    aps["x_t"] = nc.dram_tensor("x_t", [W, E, D], f32,
                                kind="ExternalInput").ap()
    aps["x_l"] = nc.dram_tensor("x_l", [W, N, D], f32,
                                kind="ExternalInput").ap()
    aps["wpack"] = nc.dram_tensor("wpack", [128, wp.n], f32,
                                  kind="ExternalInput").ap()
    aps["vpack"] = nc.dram_tensor("vpack", [128, vp.n], f32,
                                  kind="ExternalInput").ap()
    aps["bpack"] = nc.dram_tensor("bpack", [128, bpk.n], bf16,
                                  kind="ExternalInput").ap()
    aps["out"] = nc.dram_tensor("out", [W, 2 * N + E, D], f32,
                                kind="ExternalOutput").ap()
    for bname, _, _, _, _ in HBLOCKS:
        aps[f"scr_bc_{bname}"] = nc.dram_tensor(
            f"scr_bc_{bname}", [40, CBLK], bf16).ap()
        aps[f"scr_dtx_{bname}"] = nc.dram_tensor(
            f"scr_dtx_{bname}", [2, NT, 8, 16 * CT], bf16).ap()
        for ph in ("a", "f"):
            aps[f"scr_rn_{bname}_{ph}"] = nc.dram_tensor(
                f"scr_rn_{bname}_{ph}", [2, 128, NW], bf16).ap()

    with tile.TileContext(nc) as tc:
        with ExitStack() as ctx:
            _emit(ctx, tc, nc, aps, wp, vp, bpk, flags)
    nc.compile()
    return nc


_CACHE = {}


def kernel(**inputs):
    wp, vp, bpk, flags = _host_pack(inputs)
    if "prog" not in _CACHE:
        _CACHE["prog"] = _build_program(wp, vp, bpk, flags)
    nc = _CACHE["prog"]
    wpack, vpack = wp.build(), vp.build()
    bpack = bpk.build().astype(ml_dtypes.bfloat16)
    in_maps = []
    for b in range(B):
        in_maps.append({
            "x_n": np.ascontiguousarray(inputs["x_node"][b]),
            "x_t": np.ascontiguousarray(inputs["x_trace"][b]),
            "x_l": np.ascontiguousarray(inputs["x_log"][b]),
            "wpack": wpack,
            "vpack": vpack,
            "bpack": bpack,
        })
    res = run_bass_kernel_spmd(nc, in_maps, list(range(N_CORES)))
    out = np.stack([res.results[b]["out"] for b in range(B)], axis=0)
    return out.astype(np.float32)
